# revision 1
# baseline (speedup 1.0000x reference)
"""GCN (2-layer, symmetric-normalized, self-loops) on 8 TRN2 NeuronCores.

Math (reference):
    A_hat = D^-1/2 (A + I) D^-1/2        (deg over dst incl. self-loops)
    h1    = relu(A_hat @ (x @ W1) + b1)
    out   = log_softmax(A_hat @ h1 @ W2 + b2)

Device decomposition (nodes sharded by range across 8 cores, 3 launches):
    K1: ut   = bf16(dinv * (x @ W1))                     [per-core shard]
    K2: ht   = bf16(dinv*relu(dinv*((A+I) @ ut) + b1))   [gather ut table]
    K3: out  = log_softmax((dinv*((A+I) @ ht)) @ W2 + b2)
Host concatenates shard outputs between launches (index structures are
pure functions of edge_index and are built host-side).

Aggregation: edges (minus self-loops, which are added densely at drain
time) are gathered per (group-of-7-pairs x int16 source chunk) segment
with gpsimd dma_gather from a [N, 128]-padded bf16 table, then
scatter-summed into per-window PSUM accumulators via bf16 one-hot
selection matrices (is_equal against an iota row) on the tensor engine.
Per-(pair,chunk) runs are padded only to the max count over cores
(16-aligned) with valid dummy indices; blocks straddling a pair
boundary issue one matmul pair per touched pair. The gpsimd descriptor
generation (~8ns/idx) is the bottleneck; everything else hides under it.
"""

import math
import os
import sys
import types

import numpy as np
import ml_dtypes

BF16 = ml_dtypes.bfloat16

# ---------------------------------------------------------------- sizes
SMALL = bool(int(os.environ.get("BASS_GCN_SMALL", "0")))
if SMALL:
    N = 4096
    E = 32768
    CHUNK = 1024
else:
    N = 100000
    E = 1600000
    CHUNK = 32768
F_IN = 256
H = 64
C = 16
NCORE = 8
P = 128
TRACE = bool(int(os.environ.get("BASS_GCN_TRACE", "0")))

LAST_EXEC_NS = []        # per-launch exec time (filled when TRACE)


def _derived():
    ncn = N // NCORE
    padn = ((ncn + 255) // 256) * 256
    nwin = padn // P
    npair = nwin // 2
    nchunk = (N + CHUNK - 1) // CHUNK
    # groups of up to 7 pairs
    gsz = 7 if npair >= 7 else npair
    ngroup = (npair + gsz - 1) // gsz
    # padded table rows: cover every core's slab windows
    tpad = ((NCORE - 1) * ncn + padn + P - 1) // P * P
    tpad = max(tpad, N)
    return ncn, padn, nwin, npair, nchunk, gsz, ngroup, tpad


# ------------------------------------------------------- ntff shim (opt)
def _install_ntff_shim():
    try:
        if "antenv.axon_hooks" in sys.modules:
            return True
        sys.path.insert(0, "/root/.axon_site/trn_agent_boot")
        from trn_boot import _ntff_profile_via_ctypes  # type: ignore

        mod = types.ModuleType("antenv.axon_hooks")
        holder = [None]
        mod.set_axon_ntff_profile_hook = lambda h: holder.__setitem__(0, h)
        mod.get_axon_ntff_profile_hook = lambda: holder[0]
        sys.modules["antenv.axon_hooks"] = mod
        import antenv

        antenv.axon_hooks = mod
        mod.set_axon_ntff_profile_hook(
            _ntff_profile_via_ctypes("/opt/axon/libaxon_pjrt.so")
        )
        return True
    except Exception:
        return False


# ------------------------------------------------------------ host plan
def _build_plan(edge_index):
    """Index structures for the per-core edge aggregation (no self-loops).

    Edge order per core: (group, chunk, pair, stable). Per-(pair,chunk)
    runs padded to R[p,c] = 16-align(max over cores of count), with valid
    dummy idx 0 / slot 999. Segment (g,c) = concat of its pairs' runs,
    padded to a 128 multiple.

    Returns dict with:
      nseg_list [ (g,c,S) ... ]          uniform segment sizes
      blocks    [g][c] -> list of (list of (pin, col, start, stop))
      idxw      [NCORE][128, sumS/16] int16   wrapped gather indices
      slotcols  [NCORE][128, ncols] bf16      slot-in-pair per entry col
      dinv_w    [NCORE][128, nwin] f32        dinv per window column
      dinv      [N] f32
    """
    ncn, padn, nwin, npair, nchunk, gsz, ngroup, tpad = _derived()

    src_all = np.asarray(edge_index[0], np.int64)
    dst_all = np.asarray(edge_index[1], np.int64)
    # degree includes self-loop (reference: deg over dst+loop)
    deg = (np.bincount(dst_all, minlength=N) + 1).astype(np.float64)
    dinv = (1.0 / np.sqrt(deg)).astype(np.float32)

    per_core = []
    cnts = np.zeros((NCORE, npair, nchunk), np.int64)
    for c in range(NCORE):
        lo = c * ncn
        m = (dst_all >= lo) & (dst_all < lo + ncn)
        s = src_all[m]
        d = dst_all[m] - lo
        pair = d >> 8
        chunk = s // CHUNK
        grp = pair // gsz
        # sort by (group, chunk, pair), stable
        key = (grp * nchunk + chunk) * npair + pair
        order = np.argsort(key, kind="stable")
        s, d, pair, chunk = s[order], d[order], pair[order], chunk[order]
        np.add.at(cnts[c], (pair, chunk), 1)
        per_core.append((s, d, pair, chunk))

    # segment sizes: pad only the segment total to the max over cores
    # (128-aligned); per-core run boundaries float inside the segment.
    seg_cnt = np.zeros((NCORE, ngroup, nchunk), np.int64)
    for g in range(ngroup):
        p0, p1 = g * gsz, min((g + 1) * gsz, npair)
        seg_cnt[:, g, :] = cnts[:, p0:p1, :].sum(axis=1)
    seg_S = 128 * ((seg_cnt.max(axis=0) + 127) // 128)   # [ngroup, nchunk]

    total = int(seg_S.sum())

    # per-core run offsets within each segment (cumulative, unpadded) for
    # all but the last chunk; the last (tiny) chunk uses uniform padded
    # offsets so same-parity PSUM chain tags can't collide across cores.
    lastc = nchunk - 1
    Rlast = 16 * ((cnts[:, :, lastc].max(axis=0) + 15) // 16)   # [npair]
    run_off = np.zeros((NCORE, npair, nchunk), np.int64)
    for g in range(ngroup):
        p0, p1 = g * gsz, min((g + 1) * gsz, npair)
        for ch in range(nchunk):
            off = np.zeros(NCORE, np.int64)
            for p in range(p0, p1):
                run_off[:, p, ch] = off
                if ch == lastc:
                    off += int(Rlast[p])
                else:
                    off += cnts[:, p, ch]
    seg_cnt[:, :, lastc] = 0
    for g in range(ngroup):
        p0, p1 = g * gsz, min((g + 1) * gsz, npair)
        seg_cnt[:, g, lastc] = int(Rlast[p0:p1].sum())
    seg_S = 128 * ((seg_cnt.max(axis=0) + 127) // 128)
    total = int(seg_S.sum())

    # block descriptors: union over cores of pairs intersecting each block
    blocks = []
    ncols = 0
    for g in range(ngroup):
        p0, p1 = g * gsz, min((g + 1) * gsz, npair)
        gblocks = []
        touches = {p: [] for p in range(p0, p1)}
        per_ch = []
        for ch in range(nchunk):
            S = int(seg_S[g, ch])
            nb = S // 128
            ch_blocks = []
            for b in range(nb):
                lo_e, hi_e = b * 128, (b + 1) * 128
                ents = []
                for p in range(p0, p1):
                    r0 = run_off[:, p, ch]
                    r1 = r0 + cnts[:, p, ch]
                    if ((r0 < hi_e) & (r1 > lo_e)).any():
                        ents.append(p)
                        touches[p].append((ch, b))
                ch_blocks.append(ents)
            per_ch.append(ch_blocks)
        firstch, lastch = {}, {}
        for p in range(p0, p1):
            chs = sorted({ch for (ch, b) in touches[p]})
            if chs:
                firstch[p] = chs[0]
                lastch[p] = chs[-1]
        for ch in range(nchunk):
            out_blocks = []
            pblocks = {}
            for b, ents in enumerate(per_ch[ch]):
                for p in ents:
                    pblocks.setdefault(p, []).append(b)
            for b, ents in enumerate(per_ch[ch]):
                oents = []
                for p in ents:
                    start = pblocks[p][0] == b
                    stop = pblocks[p][-1] == b
                    accfirst = firstch[p] == ch
                    acclast = lastch[p] == ch
                    oents.append((p - p0, ncols, start, stop, p, accfirst,
                                  acclast))
                    ncols += 1
                out_blocks.append(oents)
            gblocks.append(out_blocks)
        blocks.append(gblocks)

    # chain-tag safety: a pair's union interval must not extend past the
    # start block of the next same-parity pair (PSUM tag reuse hazard)
    for g in range(ngroup):
        p0, p1 = g * gsz, min((g + 1) * gsz, npair)
        for ch in range(nchunk):
            lastb = {}
            firstb = {}
            for b, ents in enumerate(blocks[g][ch]):
                for (pin, col, start, stop, p, af, al) in ents:
                    firstb.setdefault(p, b)
                    lastb[p] = b
            for p in range(p0, p1 - 2):
                if p in lastb and (p + 2) in firstb:
                    assert lastb[p] <= firstb[p + 2], (g, ch, p)

    # ---- per-core data arrays
    idxw_l, slot_l, dinvw_l = [], [], []
    seg_base = {}
    off = 0
    for g in range(ngroup):
        for ch in range(nchunk):
            seg_base[(g, ch)] = off
            off += int(seg_S[g, ch])
    for c in range(NCORE):
        s, d, pair, chunk = per_core[c]
        idx16 = np.zeros(total, np.int16)
        slot = np.full(total, 999.0, np.float32)
        grp = pair // gsz
        segid = grp * nchunk + chunk
        key = segid * npair + pair
        uniq, starts = np.unique(key, return_index=True)
        for k, st in zip(uniq, starts):
            p = int(k % npair)
            sg = int(k // npair)
            g, ch = sg // nchunk, sg % nchunk
            cnt = int(cnts[c, p, ch])
            base = seg_base[(g, ch)] + int(run_off[c, p, ch])
            sl = slice(st, st + cnt)
            idx16[base:base + cnt] = (s[sl] % CHUNK).astype(np.int16)
            slot[base:base + cnt] = (d[sl] & 255).astype(np.float32)
        # wrap idx per segment: [S] -> [16, S/16] tiled to 128 rows
        cols16 = []
        for g in range(ngroup):
            for ch in range(nchunk):
                S = int(seg_S[g, ch])
                if S == 0:
                    continue
                a = seg_base[(g, ch)]
                seg = idx16[a:a + S]
                cols16.append(np.tile(seg.reshape(-1, 16).T, (8, 1)))
        idxw_l.append(np.ascontiguousarray(np.concatenate(cols16, axis=1)))

        # slot columns: one column per block entry, masked to the rows of
        # this core's run for that pair
        scols = np.full((P, ncols), 999.0, np.float32)
        for g in range(ngroup):
            p0 = g * gsz
            for ch in range(nchunk):
                a = seg_base[(g, ch)]
                for b, ents in enumerate(blocks[g][ch]):
                    blk_slots = slot[a + b * 128: a + (b + 1) * 128]
                    blk_idx_lo = b * 128
                    for (pin, col, start, stop, p_abs, accfirst,
                         acclast) in ents:
                        r0 = int(run_off[c, p_abs, ch])
                        r1 = r0 + int(cnts[c, p_abs, ch])
                        lo_i = max(r0 - blk_idx_lo, 0)
                        hi_i = min(r1 - blk_idx_lo, P)
                        if lo_i >= hi_i:
                            continue
                        colv = scols[:, col]
                        colv[lo_i:hi_i] = blk_slots[lo_i:hi_i]
        slot_l.append(scols)

        dv = np.zeros((P, nwin), np.float32)
        valid = np.arange(padn) < ncn
        dvfull = np.zeros(padn, np.float32)
        dvfull[:ncn] = dinv[c * ncn: c * ncn + ncn]
        dv[:, :] = dvfull.reshape(nwin, P).T * valid.reshape(nwin, P).T
        dinvw_l.append(dv)

    return {
        "seg_S": seg_S,
        "blocks": blocks,
        "ncols": ncols,
        "total": total,
        "idxw": idxw_l,
        "slotcols": slot_l,
        "dinv_w": dinvw_l,
        "dinv": dinv,
    }


# --------------------------------------------------------- bass builders
def _bass_mods():
    import concourse.bass as bass
    import concourse.bacc as bacc
    import concourse.tile as tile
    import concourse.mybir as mybir
    from concourse import library_config
    from concourse.masks import make_identity

    return bass, bacc, tile, mybir, library_config, make_identity


def _build_k1():
    """ut8[nw8, P, 8*H] bf16 = dinv_col * (x @ W1), 8 windows per DMA.

    Inputs: xT bf16 [F_IN, padn8], w1 bf16 [F_IN, H], dinvw f32 [P, nwin8].
    lhsT = xT tile slice [128f, 128n], rhs = w1 tile [128f, H]. Host
    unscrambles the window-packed output.
    """
    bass, bacc, tile, mybir, libcfg, make_identity = _bass_mods()
    ncn, padn, nwin, npair, nchunk, gsz, ngroup, tpad = _derived()
    nw8 = (nwin + 7) // 8
    nwin8 = nw8 * 8
    padn8 = nwin8 * P
    f32 = mybir.dt.float32
    bf16 = mybir.dt.bfloat16
    AF = mybir.ActivationFunctionType

    nc = bacc.Bacc("TRN2", target_bir_lowering=False, debug=False,
                   num_devices=NCORE)
    xT = nc.dram_tensor("xT", [F_IN, padn8], bf16, kind="ExternalInput").ap()
    w1 = nc.dram_tensor("w1", [F_IN, H], bf16, kind="ExternalInput").ap()
    dinvd = nc.dram_tensor("dinvw", [P, nwin8], f32,
                           kind="ExternalInput").ap()
    ut8 = nc.dram_tensor("ut8", [nw8 * P, 8 * H], bf16,
                         kind="ExternalOutput").ap()

    kf = F_IN // P
    with tile.TileContext(nc) as tc:
        with (
            tc.tile_pool(name="const", bufs=1) as constp,
            tc.tile_pool(name="xin", bufs=3) as xp,
            tc.tile_pool(name="ps", bufs=4, space="PSUM") as psump,
            tc.tile_pool(name="wk", bufs=3) as wp,
        ):
            w1_s = constp.tile([P, kf * H], bf16)
            for k in range(kf):
                nc.sync.dma_start(w1_s[:, k * H: (k + 1) * H],
                                  w1[k * P: (k + 1) * P, :])
            dinv_s = constp.tile([P, nwin8], f32)
            nc.sync.dma_start(dinv_s[:], dinvd[:, :])

            for g8 in range(nw8):
                xg = []
                for k in range(kf):
                    xt = xp.tile([P, 8 * P], bf16, tag=f"xt{k}")
                    nc.sync.dma_start(
                        xt[:],
                        xT[k * P: (k + 1) * P, g8 * 8 * P: (g8 + 1) * 8 * P])
                    xg.append(xt)
                obuf = wp.tile([P, 8 * H], bf16, tag="obuf")
                for j in range(8):
                    t = g8 * 8 + j
                    up = psump.tile([P, H], f32, tag="up", bufs=4)
                    for k in range(kf):
                        nc.tensor.matmul(
                            up[:], lhsT=xg[k][:, j * P: (j + 1) * P],
                            rhs=w1_s[:, k * H: (k + 1) * H],
                            start=(k == 0), stop=(k == kf - 1),
                        )
                    nc.scalar.activation(
                        obuf[:, j * H: (j + 1) * H], up[:], AF.Copy,
                        scale=dinv_s[:, t: t + 1])
                nc.sync.dma_start(ut8[g8 * P: (g8 + 1) * P, :], obuf[:])
    nc.compile()
    return nc


def _agg_core(nc, tc, mybir, plan, table, idx_s, slot_s, iota_s, identb,
              selfd, pools, drain_fn, group_end_fn=None):
    """Shared aggregation: per group, gather 4 chunk segments; per
    (pair, chunk) run a short PSUM accumulation chain (one bank per
    window, <=2 pairs pending at a block boundary); on the pair's last
    chunk append identity-matmuls of the self-loop rows before closing
    the chain; fold finished chains into SBUF accumulators (first: ACT
    copy, later: DVE add); drain per pair, then group_end_fn."""
    f32 = mybir.dt.float32
    bf16 = mybir.dt.bfloat16
    ncn, padn, nwin, npair, nchunk, gsz, ngroup, tpad = _derived()
    gatp, selp, psump, accp, slabp = pools
    seg_S = plan["seg_S"]
    blocks = plan["blocks"]

    off16 = 0
    for g in range(ngroup):
        touched = []
        seen = set()
        for ch in range(nchunk):
            for ents in blocks[g][ch]:
                for e in ents:
                    if e[0] not in seen:
                        seen.add(e[0])
                        touched.append((e[0], e[4]))
        if not touched:
            continue
        acc = {}
        for (pin, p_abs) in touched:
            acc[pin] = accp.tile([P, 2 * H], f32, tag=f"acc{pin}", bufs=2,
                                 name=f"acc{pin}")
        chain = {}
        SUBB = 16            # blocks per sub-gather (2048 idx)
        for ch in range(nchunk):
            S = int(seg_S[g, ch])
            if S == 0:
                continue
            nb = S // P
            lo = ch * CHUNK
            hi = min(tpad, (ch + 1) * CHUNK)
            subs = []
            for o in range(0, nb, SUBB):
                k = min(SUBB, nb - o)
                gsub = gatp.tile([P, k, P], bf16, tag="gat", name="gat")
                nc.gpsimd.dma_gather(
                    gsub[:],
                    table[lo:hi, :],
                    idx_s[:, off16: off16 + k * 8],
                    k * P, k * P, P, elem_step=P, single_packet=False,
                )
                off16 += k * 8
                subs.append(gsub)
            for b in range(nb):
                for (pin, col, start, stop, p_abs, accfirst, acclast) in \
                        blocks[g][ch][b]:
                    if start:
                        chain[pin] = (
                            psump.tile([P, H], f32, tag=f"cw{pin % 2}0",
                                       bufs=1, name=f"cw{pin % 2}0"),
                            psump.tile([P, H], f32, tag=f"cw{pin % 2}1",
                                       bufs=1, name=f"cw{pin % 2}1"),
                        )
                    sel2 = selp.tile([P, 2 * P], bf16, tag="sel2",
                                     name="sel2")
                    nc.vector.tensor_tensor(
                        out=sel2[:],
                        in0=slot_s[:, col: col + 1].to_broadcast([P, 2 * P]),
                        in1=iota_s[:],
                        op=mybir.AluOpType.is_equal,
                    )
                    c0, c1 = chain[pin]
                    rhs = subs[b // SUBB][:, b % SUBB, :H]
                    last_mm = stop and not acclast
                    nc.tensor.matmul(
                        c0[:], lhsT=sel2[:, :P], rhs=rhs,
                        start=start, stop=last_mm,
                    )
                    nc.tensor.matmul(
                        c1[:], lhsT=sel2[:, P:], rhs=rhs,
                        start=start, stop=last_mm,
                    )
                    if stop:
                        if acclast:
                            # append self-loop rows, closing the chains
                            for half, cx in ((0, c0), (1, c1)):
                                wi = 2 * p_abs + half
                                sl = slabp.tile([P, H], bf16, tag="sl",
                                                name="sl")
                                nc.sync.dma_start(
                                    sl[:], selfd[wi * P:(wi + 1) * P, :])
                                nc.tensor.matmul(
                                    cx[:], lhsT=identb[:], rhs=sl[:],
                                    start=False, stop=True,
                                )
                        a = acc[pin]
                        if accfirst:
                            nc.scalar.activation(
                                a[:, :H], c0[:],
                                mybir.ActivationFunctionType.Copy)
                            nc.scalar.activation(
                                a[:, H:], c1[:],
                                mybir.ActivationFunctionType.Copy)
                        else:
                            nc.vector.tensor_tensor(
                                a[:, :H], a[:, :H], c0[:],
                                op=mybir.AluOpType.add)
                            nc.vector.tensor_tensor(
                                a[:, H:], a[:, H:], c1[:],
                                op=mybir.AluOpType.add)
        for (pin, p_abs) in touched:
            a = acc[pin]
            drain_fn(pin, p_abs, a[:, :H], a[:, H:])
        if group_end_fn is not None:
            group_end_fn(g, touched)


def _agg_setup(nc, tc, mybir, libcfg, make_identity, constp, plan):
    """Load shared aggregation constants."""
    f32 = mybir.dt.float32
    bf16 = mybir.dt.bfloat16
    ncn, padn, nwin, npair, nchunk, gsz, ngroup, tpad = _derived()
    idx_cols = plan["idxw"][0].shape[1]
    ncols = plan["ncols"]

    table = nc.dram_tensor("table", [tpad, P], bf16, kind="ExternalInput").ap()
    idxd = nc.dram_tensor("idx", [P, idx_cols], mybir.dt.int16,
                          kind="ExternalInput").ap()
    slotd = nc.dram_tensor("slot", [P, ncols], f32,
                           kind="ExternalInput").ap()
    iotad = nc.dram_tensor("iota", [P, 2 * P], bf16, kind="ExternalInput").ap()
    dinvd = nc.dram_tensor("dinvw", [P, nwin], f32, kind="ExternalInput").ap()
    selfd = nc.dram_tensor("selfrows", [padn, H], bf16,
                           kind="ExternalInput").ap()

    with tc.tile_critical():
        nc.gpsimd.load_library(libcfg.mlp)
    idx_s = constp.tile([P, idx_cols], mybir.dt.int16)
    nc.sync.dma_start(idx_s[:], idxd[:, :])
    slot_s = constp.tile([P, ncols], f32)
    nc.sync.dma_start(slot_s[:], slotd[:, :])
    iota_s = constp.tile([P, 2 * P], bf16)
    nc.sync.dma_start(iota_s[:], iotad[:, :])
    dinv_s = constp.tile([P, nwin], f32)
    nc.sync.dma_start(dinv_s[:], dinvd[:, :])
    identb = constp.tile([P, P], bf16)
    make_identity(nc, identb[:])
    return table, idx_s, slot_s, iota_s, dinv_s, selfd, identb


def _build_k2(plan):
    """ht[PADN, H] bf16 = dinv*relu(dinv*(agg(ut)+self) + b1) per core."""
    bass, bacc, tile, mybir, libcfg, make_identity = _bass_mods()
    ncn, padn, nwin, npair, nchunk, gsz, ngroup, tpad = _derived()
    f32 = mybir.dt.float32
    bf16 = mybir.dt.bfloat16
    AF = mybir.ActivationFunctionType

    nc = bacc.Bacc("TRN2", target_bir_lowering=False, debug=False,
                   num_devices=NCORE)
    b1d = nc.dram_tensor("b1rep", [P, H], f32, kind="ExternalInput").ap()
    ht = nc.dram_tensor("ht", [padn, H], bf16, kind="ExternalOutput").ap()

    with tile.TileContext(nc) as tc:
        with (
            tc.tile_pool(name="const", bufs=1) as constp,
            tc.tile_pool(name="gat", bufs=10) as gatp,
            tc.tile_pool(name="sel", bufs=12) as selp,
            tc.tile_pool(name="slab", bufs=8) as slabp,
            tc.tile_pool(name="ps", bufs=1, space="PSUM") as psump,
            tc.tile_pool(name="acc", bufs=2) as accp,
            tc.tile_pool(name="wk", bufs=6) as wp,
        ):
            table, idx_s, slot_s, iota_s, dinv_s, selfd, identb = _agg_setup(
                nc, tc, mybir, libcfg, make_identity, constp, plan)
            b1_s = constp.tile([P, H], f32)
            nc.sync.dma_start(b1_s[:], b1d[:, :])

            def drain(pin, pr, a0, a1):
                for wi, a in ((2 * pr, a0), (2 * pr + 1, a1)):
                    t2 = wp.tile([P, H], f32, tag="t2", name="t2")
                    nc.scalar.activation(t2[:], a, AF.Copy,
                                         scale=dinv_s[:, wi: wi + 1])
                    t3 = wp.tile([P, H], f32, tag="t3", name="t3")
                    nc.vector.tensor_tensor(
                        t3[:], t2[:], b1_s[:], op=mybir.AluOpType.add)
                    t4 = wp.tile([P, H], bf16, tag="t4", name="t4")
                    nc.scalar.activation(t4[:], t3[:], AF.Relu,
                                         scale=dinv_s[:, wi: wi + 1])
                    nc.sync.dma_start(ht[wi * P: (wi + 1) * P, :], t4[:])

            _agg_core(nc, tc, mybir, plan, table, idx_s, slot_s, iota_s,
                      identb, selfd, (gatp, selp, psump, accp, slabp), drain)
    nc.compile()
    return nc


def _build_k3(plan):
    """out[PADN, C] f32 = log_softmax((dinv*(agg(ht)+self)) @ W2 + b2).

    Ln is batched per group (one [P, n_windows] Ln) to avoid per-window
    Exp<->Ln activation-table reloads.
    """
    bass, bacc, tile, mybir, libcfg, make_identity = _bass_mods()
    ncn, padn, nwin, npair, nchunk, gsz, ngroup, tpad = _derived()
    f32 = mybir.dt.float32
    bf16 = mybir.dt.bfloat16
    AF = mybir.ActivationFunctionType

    nc = bacc.Bacc("TRN2", target_bir_lowering=False, debug=False,
                   num_devices=NCORE)
    w2d = nc.dram_tensor("w2", [H, C], f32, kind="ExternalInput").ap()
    b2d = nc.dram_tensor("b2rep", [P, C], f32, kind="ExternalInput").ap()
    outd = nc.dram_tensor("out", [padn, C], f32, kind="ExternalOutput").ap()

    with tile.TileContext(nc) as tc:
        with (
            tc.tile_pool(name="const", bufs=1) as constp,
            tc.tile_pool(name="gat", bufs=10) as gatp,
            tc.tile_pool(name="sel", bufs=12) as selp,
            tc.tile_pool(name="slab", bufs=8) as slabp,
            tc.tile_pool(name="ps", bufs=1, space="PSUM") as psump,
            tc.tile_pool(name="acc", bufs=2) as accp,
            tc.tile_pool(name="wk", bufs=6) as wp,
            tc.tile_pool(name="grp", bufs=2) as grp,
        ):
            table, idx_s, slot_s, iota_s, dinv_s, selfd, identb = _agg_setup(
                nc, tc, mybir, libcfg, make_identity, constp, plan)
            w2_s = constp.tile([H, C], f32)
            nc.sync.dma_start(w2_s[:], w2d[:, :])
            b2_s = constp.tile([P, C], f32)
            nc.sync.dma_start(b2_s[:], b2d[:, :])
            ident = constp.tile([P, P], f32)
            make_identity(nc, ident[:])

            nwg = 2 * gsz
            gtiles = [None]

            def drain(pin, pr, a0, a1):
                if gtiles[0] is None:
                    gtiles[0] = (
                        grp.tile([P, nwg * C], f32, tag="zbuf", name="zbuf"),
                        grp.tile([P, nwg], f32, tag="negm", name="negm"),
                        grp.tile([P, nwg], f32, tag="sa", name="sa"),
                    )
                zbuf, negm_g, sa_g = gtiles[0]
                for half, a in ((0, a0), (1, a1)):
                    wi = 2 * pr + half
                    wg = 2 * pin + half
                    t1 = wp.tile([P, H], f32, tag="t1", name="t1")
                    nc.scalar.activation(t1[:], a, AF.Copy,
                                         scale=dinv_s[:, wi: wi + 1])
                    t1T_p = psump.tile([H, P], f32, tag="t1T", bufs=1,
                                       name="t1T")
                    nc.tensor.transpose(t1T_p[:], t1[:], ident[:])
                    t1T = wp.tile([H, P], f32, tag="t1Ts", name="t1Ts")
                    nc.scalar.activation(t1T[:], t1T_p[:], AF.Copy)
                    yT_p = psump.tile([C, P], f32, tag="yT", bufs=1,
                                      name="yT")
                    nc.tensor.matmul(yT_p[:], lhsT=w2_s[:], rhs=t1T[:],
                                     start=True, stop=True)
                    yT = wp.tile([C, P], f32, tag="yTs", name="yTs")
                    nc.scalar.activation(yT[:], yT_p[:], AF.Copy)
                    y_p = psump.tile([P, C], f32, tag="y", bufs=1, name="y")
                    nc.tensor.transpose(y_p[:], yT[:], ident[:C, :C])
                    z = zbuf[:, wg * C: (wg + 1) * C]
                    nc.vector.tensor_tensor(z, y_p[:], b2_s[:],
                                            op=mybir.AluOpType.add)
                    nc.vector.tensor_reduce(
                        negm_g[:, wg: wg + 1], z, axis=mybir.AxisListType.X,
                        op=mybir.AluOpType.max, negate=True,
                    )
                    e = wp.tile([P, C], f32, tag="e", name="e")
                    nc.scalar.activation(
                        e[:], z, AF.Exp,
                        bias=negm_g[:, wg: wg + 1],
                        accum_out=sa_g[:, wg: wg + 1],
                    )

            def group_end(g, touched):
                zbuf, negm_g, sa_g = gtiles[0]
                gtiles[0] = None
                lns = wp.tile([P, nwg], f32, tag="lns", name="lns")
                nc.scalar.activation(lns[:], sa_g[:], AF.Ln)
                for (pin, pr) in touched:
                    for half in (0, 1):
                        wi = 2 * pr + half
                        wg = 2 * pin + half
                        o = wp.tile([P, C], f32, tag="o", name="o")
                        nc.vector.tensor_scalar(
                            out=o[:], in0=zbuf[:, wg * C: (wg + 1) * C],
                            scalar1=negm_g[:, wg: wg + 1],
                            scalar2=lns[:, wg: wg + 1],
                            op0=mybir.AluOpType.add,
                            op1=mybir.AluOpType.subtract,
                        )
                        nc.sync.dma_start(outd[wi * P: (wi + 1) * P, :],
                                          o[:])

            _agg_core(nc, tc, mybir, plan, table, idx_s, slot_s, iota_s,
                      identb, selfd, (gatp, selp, psump, accp, slabp), drain,
                      group_end)
    nc.compile()
    return nc


def _run(nc, in_maps):
    if os.environ.get("BASS_GCN_SIM"):
        from concourse.bass_interp import MultiCoreSim

        sim = MultiCoreSim(nc, num_cores=NCORE, trace=False)
        for c in range(NCORE):
            for k, v in in_maps[c].items():
                sim.cores[c].tensor(k)[:] = v
        sim.simulate()
        outs = []
        for c in range(NCORE):
            names = [
                a.memorylocations[0].name
                for a in nc.m.functions[0].allocations
                if getattr(a, "kind", None) == "ExternalOutput"
            ]
            outs.append({n: np.array(sim.cores[c].tensor(n)) for n in names})
        return outs

    from concourse.bass_utils import run_bass_kernel_spmd

    trace = TRACE and _install_ntff_shim()
    res = run_bass_kernel_spmd(nc, in_maps, core_ids=list(range(NCORE)),
                               trace=trace)
    if res.exec_time_ns:
        LAST_EXEC_NS.append(res.exec_time_ns)
    return res.results


# ---------------------------------------------------------------- kernel
def kernel(x, edge_index, W1, b1, W2, b2):
    ncn, padn, nwin, npair, nchunk, gsz, ngroup, tpad = _derived()
    LAST_EXEC_NS.clear()

    x = np.asarray(x, np.float32)
    edge_index = np.asarray(edge_index)
    W1 = np.asarray(W1, np.float32)
    b1 = np.asarray(b1, np.float32)
    W2 = np.asarray(W2, np.float32)
    b2 = np.asarray(b2, np.float32)

    plan = _build_plan(edge_index)

    iota2 = np.tile(np.arange(2 * P, dtype=np.float32)[None, :],
                    (P, 1)).astype(BF16)
    b1rep = np.tile(b1[None, :], (P, 1)).astype(np.float32)
    b2rep = np.tile(b2[None, :], (P, 1)).astype(np.float32)

    # ---- K1
    nw8 = (nwin + 7) // 8
    nwin8 = nw8 * 8
    padn8 = nwin8 * P
    nc1 = _build_k1()
    in1 = []
    for c in range(NCORE):
        xc = np.zeros((padn8, F_IN), np.float32)
        xc[:ncn] = x[c * ncn: (c + 1) * ncn]
        dv8 = np.zeros((P, nwin8), np.float32)
        dv8[:, :nwin] = plan["dinv_w"][c]
        in1.append({
            "xT": np.ascontiguousarray(xc.T).astype(BF16),
            "w1": W1.astype(BF16),
            "dinvw": dv8,
        })
    r1 = _run(nc1, in1)
    ut = np.concatenate([
        np.asarray(r1[c]["ut8"]).reshape(nw8, P, 8, H)
        .transpose(0, 2, 1, 3).reshape(padn8, H)[:ncn]
        for c in range(NCORE)
    ], axis=0)

    def padded_table(t):
        tab = np.zeros((tpad, P), BF16)
        tab[:N, :H] = t
        return tab

    def self_rows(t, c):
        sr = np.zeros((padn, H), BF16)
        lo = c * ncn
        sr[:ncn] = t[lo: lo + ncn]
        return sr

    # ---- K2
    nc2 = _build_k2(plan)
    tab2 = padded_table(ut)
    in2 = [{
        "table": tab2,
        "idx": plan["idxw"][c],
        "slot": plan["slotcols"][c],
        "iota": iota2,
        "dinvw": plan["dinv_w"][c],
        "selfrows": self_rows(ut, c),
        "b1rep": b1rep,
    } for c in range(NCORE)]
    r2 = _run(nc2, in2)
    ht = np.concatenate([r2[c]["ht"][:ncn] for c in range(NCORE)], axis=0)

    # ---- K3
    nc3 = _build_k3(plan)
    tab3 = padded_table(ht)
    in3 = [{
        "table": tab3,
        "idx": plan["idxw"][c],
        "slot": plan["slotcols"][c],
        "iota": iota2,
        "dinvw": plan["dinv_w"][c],
        "selfrows": self_rows(ht, c),
        "w2": W2,
        "b2rep": b2rep,
    } for c in range(NCORE)]
    r3 = _run(nc3, in3)
    out = np.concatenate([r3[c]["out"][:ncn] for c in range(NCORE)], axis=0)
    return np.ascontiguousarray(out.astype(np.float32))



# revision 2
# speedup vs baseline: 1.6948x; 1.6948x over previous
"""GCN (2-layer, symmetric-normalized, self-loops) on 8 TRN2 NeuronCores.

Math (reference):
    A_hat = D^-1/2 (A + I) D^-1/2        (deg over dst incl. self-loops)
    h1    = relu(A_hat @ (x @ W1) + b1)
    out   = log_softmax(A_hat @ h1 @ W2 + b2)

Device decomposition (nodes sharded by range across 8 cores, 3 launches):
    K1: ut   = bf16(dinv * (x @ W1))                     [per-core shard]
    K2: ht   = bf16(dinv*relu(dinv*((A+I) @ ut) + b1))   [gather ut table]
    K3: out  = log_softmax((dinv*((A+I) @ ht)) @ W2 + b2)
Host concatenates shard outputs between launches (index structures are
pure functions of edge_index and are built host-side).

Aggregation: edges (minus self-loops, which are added densely at drain
time) are gathered per (group-of-7-pairs x int16 source chunk) segment
with gpsimd dma_gather from a [N, 128]-padded bf16 table, then
scatter-summed into per-window PSUM accumulators via bf16 one-hot
selection matrices (is_equal against an iota row) on the tensor engine.
Per-(pair,chunk) runs are padded only to the max count over cores
(16-aligned) with valid dummy indices; blocks straddling a pair
boundary issue one matmul pair per touched pair. The gpsimd descriptor
generation (~8ns/idx) is the bottleneck; everything else hides under it.
"""

import math
import os
import sys
import types

import numpy as np
import ml_dtypes

BF16 = ml_dtypes.bfloat16

# ---------------------------------------------------------------- sizes
SMALL = bool(int(os.environ.get("BASS_GCN_SMALL", "0")))
if SMALL:
    N = 4096
    E = 32768
    CHUNK = 1024
else:
    N = 100000
    E = 1600000
    CHUNK = 32768
F_IN = 256
H = 64
C = 16
NCORE = 8
P = 128
TRACE = bool(int(os.environ.get("BASS_GCN_TRACE", "0")))

LAST_EXEC_NS = []        # per-launch exec time (filled when TRACE)


def _derived():
    ncn = N // NCORE
    padn = ((ncn + 255) // 256) * 256
    nwin = padn // P
    npair = nwin // 2
    nchunk = (N + CHUNK - 1) // CHUNK
    # groups of up to 7 pairs
    gsz = 7 if npair >= 7 else npair
    ngroup = (npair + gsz - 1) // gsz
    # padded table rows: cover every core's slab windows
    tpad = ((NCORE - 1) * ncn + padn + P - 1) // P * P
    tpad = max(tpad, N)
    return ncn, padn, nwin, npair, nchunk, gsz, ngroup, tpad


# ------------------------------------------------------- ntff shim (opt)
def _install_ntff_shim():
    try:
        if "antenv.axon_hooks" in sys.modules:
            return True
        sys.path.insert(0, "/root/.axon_site/trn_agent_boot")
        from trn_boot import _ntff_profile_via_ctypes  # type: ignore

        mod = types.ModuleType("antenv.axon_hooks")
        holder = [None]
        mod.set_axon_ntff_profile_hook = lambda h: holder.__setitem__(0, h)
        mod.get_axon_ntff_profile_hook = lambda: holder[0]
        sys.modules["antenv.axon_hooks"] = mod
        import antenv

        antenv.axon_hooks = mod
        mod.set_axon_ntff_profile_hook(
            _ntff_profile_via_ctypes("/opt/axon/libaxon_pjrt.so")
        )
        return True
    except Exception:
        return False


# ------------------------------------------------------------ host plan
def _build_plan(edge_index):
    """Index structures for the per-core edge aggregation (no self-loops).

    Edge order per core: (group, chunk, pair, stable). Per-(pair,chunk)
    runs padded to R[p,c] = 16-align(max over cores of count), with valid
    dummy idx 0 / slot 999. Segment (g,c) = concat of its pairs' runs,
    padded to a 128 multiple.

    Returns dict with:
      nseg_list [ (g,c,S) ... ]          uniform segment sizes
      blocks    [g][c] -> list of (list of (pin, col, start, stop))
      idxw      [NCORE][128, sumS/16] int16   wrapped gather indices
      slotcols  [NCORE][128, ncols] bf16      slot-in-pair per entry col
      dinv_w    [NCORE][128, nwin] f32        dinv per window column
      dinv      [N] f32
    """
    ncn, padn, nwin, npair, nchunk, gsz, ngroup, tpad = _derived()

    src_all = np.asarray(edge_index[0], np.int64)
    dst_all = np.asarray(edge_index[1], np.int64)
    # degree includes self-loop (reference: deg over dst+loop)
    deg = (np.bincount(dst_all, minlength=N) + 1).astype(np.float64)
    dinv = (1.0 / np.sqrt(deg)).astype(np.float32)

    per_core = []
    cnts = np.zeros((NCORE, npair, nchunk), np.int64)
    for c in range(NCORE):
        lo = c * ncn
        m = (dst_all >= lo) & (dst_all < lo + ncn)
        s = src_all[m]
        d = dst_all[m] - lo
        pair = d >> 8
        chunk = s // CHUNK
        grp = pair // gsz
        # sort by (group, chunk, pair), stable
        key = (grp * nchunk + chunk) * npair + pair
        order = np.argsort(key, kind="stable")
        s, d, pair, chunk = s[order], d[order], pair[order], chunk[order]
        np.add.at(cnts[c], (pair, chunk), 1)
        per_core.append((s, d, pair, chunk))

    # segment sizes: pad only the segment total to the max over cores
    # (128-aligned); per-core run boundaries float inside the segment.
    seg_cnt = np.zeros((NCORE, ngroup, nchunk), np.int64)
    for g in range(ngroup):
        p0, p1 = g * gsz, min((g + 1) * gsz, npair)
        seg_cnt[:, g, :] = cnts[:, p0:p1, :].sum(axis=1)
    seg_S = 128 * ((seg_cnt.max(axis=0) + 127) // 128)   # [ngroup, nchunk]

    total = int(seg_S.sum())

    # per-core run offsets within each segment (cumulative, unpadded) for
    # all but the last chunk; the last (tiny) chunk uses uniform padded
    # offsets so same-parity PSUM chain tags can't collide across cores.
    lastc = nchunk - 1
    Rlast = 16 * ((cnts[:, :, lastc].max(axis=0) + 15) // 16)   # [npair]
    run_off = np.zeros((NCORE, npair, nchunk), np.int64)
    for g in range(ngroup):
        p0, p1 = g * gsz, min((g + 1) * gsz, npair)
        for ch in range(nchunk):
            off = np.zeros(NCORE, np.int64)
            for p in range(p0, p1):
                run_off[:, p, ch] = off
                if ch == lastc:
                    off += int(Rlast[p])
                else:
                    off += cnts[:, p, ch]
    seg_cnt[:, :, lastc] = 0
    for g in range(ngroup):
        p0, p1 = g * gsz, min((g + 1) * gsz, npair)
        seg_cnt[:, g, lastc] = int(Rlast[p0:p1].sum())
    seg_S = 128 * ((seg_cnt.max(axis=0) + 127) // 128)
    total = int(seg_S.sum())

    # block descriptors: union over cores of pairs intersecting each block
    blocks = []
    ncols = 0
    for g in range(ngroup):
        p0, p1 = g * gsz, min((g + 1) * gsz, npair)
        gblocks = []
        touches = {p: [] for p in range(p0, p1)}
        per_ch = []
        for ch in range(nchunk):
            S = int(seg_S[g, ch])
            nb = S // 128
            ch_blocks = []
            for b in range(nb):
                lo_e, hi_e = b * 128, (b + 1) * 128
                ents = []
                for p in range(p0, p1):
                    r0 = run_off[:, p, ch]
                    r1 = r0 + cnts[:, p, ch]
                    if ((r0 < hi_e) & (r1 > lo_e)).any():
                        ents.append(p)
                        touches[p].append((ch, b))
                ch_blocks.append(ents)
            per_ch.append(ch_blocks)
        firstch, lastch = {}, {}
        for p in range(p0, p1):
            chs = sorted({ch for (ch, b) in touches[p]})
            if chs:
                firstch[p] = chs[0]
                lastch[p] = chs[-1]
        for ch in range(nchunk):
            out_blocks = []
            pblocks = {}
            for b, ents in enumerate(per_ch[ch]):
                for p in ents:
                    pblocks.setdefault(p, []).append(b)
            for b, ents in enumerate(per_ch[ch]):
                oents = []
                for p in ents:
                    start = pblocks[p][0] == b
                    stop = pblocks[p][-1] == b
                    accfirst = firstch[p] == ch
                    acclast = lastch[p] == ch
                    oents.append((p - p0, ncols, start, stop, p, accfirst,
                                  acclast))
                    ncols += 1
                out_blocks.append(oents)
            gblocks.append(out_blocks)
        blocks.append(gblocks)

    # chain-tag safety: a pair's union interval must not extend past the
    # start block of the next same-parity pair (PSUM tag reuse hazard)
    for g in range(ngroup):
        p0, p1 = g * gsz, min((g + 1) * gsz, npair)
        for ch in range(nchunk):
            lastb = {}
            firstb = {}
            for b, ents in enumerate(blocks[g][ch]):
                for (pin, col, start, stop, p, af, al) in ents:
                    firstb.setdefault(p, b)
                    lastb[p] = b
            for p in range(p0, p1 - 2):
                if p in lastb and (p + 2) in firstb:
                    assert lastb[p] <= firstb[p + 2], (g, ch, p)

    # ---- per-core data arrays
    idxw_l, slot_l, dinvw_l = [], [], []
    seg_base = {}
    off = 0
    for g in range(ngroup):
        for ch in range(nchunk):
            seg_base[(g, ch)] = off
            off += int(seg_S[g, ch])
    for c in range(NCORE):
        s, d, pair, chunk = per_core[c]
        idx16 = np.zeros(total, np.int16)
        slot = np.full(total, 999.0, np.float32)
        grp = pair // gsz
        segid = grp * nchunk + chunk
        key = segid * npair + pair
        uniq, starts = np.unique(key, return_index=True)
        for k, st in zip(uniq, starts):
            p = int(k % npair)
            sg = int(k // npair)
            g, ch = sg // nchunk, sg % nchunk
            cnt = int(cnts[c, p, ch])
            base = seg_base[(g, ch)] + int(run_off[c, p, ch])
            sl = slice(st, st + cnt)
            idx16[base:base + cnt] = (s[sl] % CHUNK).astype(np.int16)
            slot[base:base + cnt] = (d[sl] & 255).astype(np.float32)
        # wrap idx per segment: [S] -> [16, S/16] tiled to 128 rows
        cols16 = []
        for g in range(ngroup):
            for ch in range(nchunk):
                S = int(seg_S[g, ch])
                if S == 0:
                    continue
                a = seg_base[(g, ch)]
                seg = idx16[a:a + S]
                cols16.append(np.tile(seg.reshape(-1, 16).T, (8, 1)))
        idxw_l.append(np.ascontiguousarray(np.concatenate(cols16, axis=1)))

        # slot columns: one column per block entry, masked to the rows of
        # this core's run for that pair
        scols = np.full((P, ncols), 999.0, np.float32)
        for g in range(ngroup):
            p0 = g * gsz
            for ch in range(nchunk):
                a = seg_base[(g, ch)]
                for b, ents in enumerate(blocks[g][ch]):
                    blk_slots = slot[a + b * 128: a + (b + 1) * 128]
                    blk_idx_lo = b * 128
                    for (pin, col, start, stop, p_abs, accfirst,
                         acclast) in ents:
                        r0 = int(run_off[c, p_abs, ch])
                        r1 = r0 + int(cnts[c, p_abs, ch])
                        lo_i = max(r0 - blk_idx_lo, 0)
                        hi_i = min(r1 - blk_idx_lo, P)
                        if lo_i >= hi_i:
                            continue
                        colv = scols[:, col]
                        colv[lo_i:hi_i] = blk_slots[lo_i:hi_i]
        slot_l.append(scols)

        dv = np.zeros((P, nwin), np.float32)
        valid = np.arange(padn) < ncn
        dvfull = np.zeros(padn, np.float32)
        dvfull[:ncn] = dinv[c * ncn: c * ncn + ncn]
        dv[:, :] = dvfull.reshape(nwin, P).T * valid.reshape(nwin, P).T
        dinvw_l.append(dv)

    return {
        "seg_S": seg_S,
        "blocks": blocks,
        "ncols": ncols,
        "total": total,
        "idxw": idxw_l,
        "slotcols": slot_l,
        "dinv_w": dinvw_l,
        "dinv": dinv,
    }


# --------------------------------------------------------- bass builders
def _bass_mods():
    import concourse.bass as bass
    import concourse.bacc as bacc
    import concourse.tile as tile
    import concourse.mybir as mybir
    from concourse import library_config
    from concourse.masks import make_identity

    return bass, bacc, tile, mybir, library_config, make_identity


def _build_k1():
    """ut8[nw8, P, 8*H] bf16 = dinv_col * (x @ W1), 8 windows per DMA.

    Inputs: xT bf16 [F_IN, padn8], w1 bf16 [F_IN, H], dinvw f32 [P, nwin8].
    lhsT = xT tile slice [128f, 128n], rhs = w1 tile [128f, H]. Host
    unscrambles the window-packed output.
    """
    bass, bacc, tile, mybir, libcfg, make_identity = _bass_mods()
    ncn, padn, nwin, npair, nchunk, gsz, ngroup, tpad = _derived()
    nw8 = (nwin + 7) // 8
    nwin8 = nw8 * 8
    padn8 = nwin8 * P
    f32 = mybir.dt.float32
    bf16 = mybir.dt.bfloat16
    AF = mybir.ActivationFunctionType

    nc = bacc.Bacc("TRN2", target_bir_lowering=False, debug=False,
                   num_devices=NCORE)
    xT = nc.dram_tensor("xT", [F_IN, padn8], bf16, kind="ExternalInput").ap()
    w1 = nc.dram_tensor("w1", [F_IN, H], bf16, kind="ExternalInput").ap()
    dinvd = nc.dram_tensor("dinvw", [P, nwin8], f32,
                           kind="ExternalInput").ap()
    ut8 = nc.dram_tensor("ut8", [nw8 * P, 8 * H], bf16,
                         kind="ExternalOutput").ap()

    kf = F_IN // P
    with tile.TileContext(nc) as tc:
        with (
            tc.tile_pool(name="const", bufs=1) as constp,
            tc.tile_pool(name="xin", bufs=3) as xp,
            tc.tile_pool(name="ps", bufs=4, space="PSUM") as psump,
            tc.tile_pool(name="wk", bufs=3) as wp,
        ):
            w1_s = constp.tile([P, kf * H], bf16)
            for k in range(kf):
                nc.sync.dma_start(w1_s[:, k * H: (k + 1) * H],
                                  w1[k * P: (k + 1) * P, :])
            dinv_s = constp.tile([P, nwin8], f32)
            nc.sync.dma_start(dinv_s[:], dinvd[:, :])

            for g8 in range(nw8):
                xg = []
                for k in range(kf):
                    xt = xp.tile([P, 8 * P], bf16, tag=f"xt{k}")
                    nc.sync.dma_start(
                        xt[:],
                        xT[k * P: (k + 1) * P, g8 * 8 * P: (g8 + 1) * 8 * P])
                    xg.append(xt)
                obuf = wp.tile([P, 8 * H], bf16, tag="obuf")
                for j in range(8):
                    t = g8 * 8 + j
                    up = psump.tile([P, H], f32, tag="up", bufs=4)
                    for k in range(kf):
                        nc.tensor.matmul(
                            up[:], lhsT=xg[k][:, j * P: (j + 1) * P],
                            rhs=w1_s[:, k * H: (k + 1) * H],
                            start=(k == 0), stop=(k == kf - 1),
                        )
                    nc.scalar.activation(
                        obuf[:, j * H: (j + 1) * H], up[:], AF.Copy,
                        scale=dinv_s[:, t: t + 1])
                nc.sync.dma_start(ut8[g8 * P: (g8 + 1) * P, :], obuf[:])
    nc.compile()
    return nc


def _agg_core(nc, tc, mybir, plan, table, idx_s, slot_s, iota_s, identb,
              selfd, pools, drain_fn, group_end_fn=None):
    """Shared aggregation: per group, gather 4 chunk segments; per
    (pair, chunk) run a short PSUM accumulation chain (one bank per
    window, <=2 pairs pending at a block boundary); on the pair's last
    chunk append identity-matmuls of the self-loop rows before closing
    the chain; fold finished chains into SBUF accumulators (first: ACT
    copy, later: DVE add); drain per pair, then group_end_fn."""
    f32 = mybir.dt.float32
    bf16 = mybir.dt.bfloat16
    ncn, padn, nwin, npair, nchunk, gsz, ngroup, tpad = _derived()
    gatp, selp, psump, accp, slabp = pools
    seg_S = plan["seg_S"]
    blocks = plan["blocks"]

    off16 = 0
    for g in range(ngroup):
        touched = []
        seen = set()
        for ch in range(nchunk):
            for ents in blocks[g][ch]:
                for e in ents:
                    if e[0] not in seen:
                        seen.add(e[0])
                        touched.append((e[0], e[4]))
        if not touched:
            continue
        acc = {}
        for (pin, p_abs) in touched:
            acc[pin] = accp.tile([P, 2 * H], f32, tag=f"acc{pin}", bufs=2,
                                 name=f"acc{pin}")
        chain = {}
        SUBB = 32            # blocks per sub-gather (4096 idx)
        for ch in range(nchunk):
            S = int(seg_S[g, ch])
            if S == 0:
                continue
            nb = S // P
            lo = ch * CHUNK
            hi = min(tpad, (ch + 1) * CHUNK)
            subs = []
            for o in range(0, nb, SUBB):
                k = min(SUBB, nb - o)
                gsub = gatp.tile([P, k, P], bf16, tag="gat", name="gat")
                nc.gpsimd.dma_gather(
                    gsub[:],
                    table[lo:hi, :],
                    idx_s[:, off16: off16 + k * 8],
                    k * P, k * P, P, elem_step=P, single_packet=False,
                )
                off16 += k * 8
                subs.append(gsub)
            for b in range(nb):
                for (pin, col, start, stop, p_abs, accfirst, acclast) in \
                        blocks[g][ch][b]:
                    if start:
                        chain[pin] = (
                            psump.tile([P, H], f32, tag=f"cw{pin % 2}0",
                                       bufs=1, name=f"cw{pin % 2}0"),
                            psump.tile([P, H], f32, tag=f"cw{pin % 2}1",
                                       bufs=1, name=f"cw{pin % 2}1"),
                        )
                    sel2 = selp.tile([P, 2 * P], bf16, tag="sel2",
                                     name="sel2")
                    nc.vector.tensor_tensor(
                        out=sel2[:],
                        in0=slot_s[:, col: col + 1].to_broadcast([P, 2 * P]),
                        in1=iota_s[:],
                        op=mybir.AluOpType.is_equal,
                    )
                    c0, c1 = chain[pin]
                    rhs = subs[b // SUBB][:, b % SUBB, :H]
                    last_mm = stop and not acclast
                    nc.tensor.matmul(
                        c0[:], lhsT=sel2[:, :P], rhs=rhs,
                        start=start, stop=last_mm,
                    )
                    nc.tensor.matmul(
                        c1[:], lhsT=sel2[:, P:], rhs=rhs,
                        start=start, stop=last_mm,
                    )
                    if stop:
                        if acclast:
                            # append self-loop rows, closing the chains
                            for half, cx in ((0, c0), (1, c1)):
                                wi = 2 * p_abs + half
                                sl = slabp.tile([P, H], bf16, tag="sl",
                                                name="sl")
                                nc.sync.dma_start(
                                    sl[:], selfd[wi * P:(wi + 1) * P, :])
                                nc.tensor.matmul(
                                    cx[:], lhsT=identb[:], rhs=sl[:],
                                    start=False, stop=True,
                                )
                        a = acc[pin]
                        if accfirst:
                            nc.scalar.activation(
                                a[:, :H], c0[:],
                                mybir.ActivationFunctionType.Copy)
                            nc.scalar.activation(
                                a[:, H:], c1[:],
                                mybir.ActivationFunctionType.Copy)
                        else:
                            nc.vector.tensor_tensor(
                                a[:, :H], a[:, :H], c0[:],
                                op=mybir.AluOpType.add)
                            nc.vector.tensor_tensor(
                                a[:, H:], a[:, H:], c1[:],
                                op=mybir.AluOpType.add)
        for (pin, p_abs) in touched:
            a = acc[pin]
            drain_fn(pin, p_abs, a[:, :H], a[:, H:])
        if group_end_fn is not None:
            group_end_fn(g, touched)


def _agg_setup(nc, tc, mybir, libcfg, make_identity, constp, plan):
    """Load shared aggregation constants."""
    f32 = mybir.dt.float32
    bf16 = mybir.dt.bfloat16
    ncn, padn, nwin, npair, nchunk, gsz, ngroup, tpad = _derived()
    idx_cols = plan["idxw"][0].shape[1]
    ncols = plan["ncols"]

    table = nc.dram_tensor("table", [tpad, P], bf16, kind="ExternalInput").ap()
    idxd = nc.dram_tensor("idx", [P, idx_cols], mybir.dt.int16,
                          kind="ExternalInput").ap()
    slotd = nc.dram_tensor("slot", [P, ncols], f32,
                           kind="ExternalInput").ap()
    iotad = nc.dram_tensor("iota", [P, 2 * P], bf16, kind="ExternalInput").ap()
    dinvd = nc.dram_tensor("dinvw", [P, nwin], f32, kind="ExternalInput").ap()
    selfd = nc.dram_tensor("selfrows", [padn, H], bf16,
                           kind="ExternalInput").ap()

    with tc.tile_critical():
        nc.gpsimd.load_library(libcfg.mlp)
    idx_s = constp.tile([P, idx_cols], mybir.dt.int16)
    nc.sync.dma_start(idx_s[:], idxd[:, :])
    slot_s = constp.tile([P, ncols], f32)
    nc.sync.dma_start(slot_s[:], slotd[:, :])
    iota_s = constp.tile([P, 2 * P], bf16)
    nc.sync.dma_start(iota_s[:], iotad[:, :])
    dinv_s = constp.tile([P, nwin], f32)
    nc.sync.dma_start(dinv_s[:], dinvd[:, :])
    identb = constp.tile([P, P], bf16)
    make_identity(nc, identb[:])
    return table, idx_s, slot_s, iota_s, dinv_s, selfd, identb


def _build_k2(plan):
    """ht[PADN, H] bf16 = dinv*relu(dinv*(agg(ut)+self) + b1) per core."""
    bass, bacc, tile, mybir, libcfg, make_identity = _bass_mods()
    ncn, padn, nwin, npair, nchunk, gsz, ngroup, tpad = _derived()
    f32 = mybir.dt.float32
    bf16 = mybir.dt.bfloat16
    AF = mybir.ActivationFunctionType

    nc = bacc.Bacc("TRN2", target_bir_lowering=False, debug=False,
                   num_devices=NCORE)
    b1d = nc.dram_tensor("b1rep", [P, H], f32, kind="ExternalInput").ap()
    ht = nc.dram_tensor("ht", [padn, H], bf16, kind="ExternalOutput").ap()

    with tile.TileContext(nc) as tc:
        with (
            tc.tile_pool(name="const", bufs=1) as constp,
            tc.tile_pool(name="gat", bufs=6) as gatp,
            tc.tile_pool(name="sel", bufs=12) as selp,
            tc.tile_pool(name="slab", bufs=8) as slabp,
            tc.tile_pool(name="ps", bufs=1, space="PSUM") as psump,
            tc.tile_pool(name="acc", bufs=2) as accp,
            tc.tile_pool(name="wk", bufs=6) as wp,
        ):
            table, idx_s, slot_s, iota_s, dinv_s, selfd, identb = _agg_setup(
                nc, tc, mybir, libcfg, make_identity, constp, plan)
            b1_s = constp.tile([P, H], f32)
            nc.sync.dma_start(b1_s[:], b1d[:, :])

            def drain(pin, pr, a0, a1):
                for wi, a in ((2 * pr, a0), (2 * pr + 1, a1)):
                    t2 = wp.tile([P, H], f32, tag="t2", name="t2")
                    nc.scalar.activation(t2[:], a, AF.Copy,
                                         scale=dinv_s[:, wi: wi + 1])
                    t3 = wp.tile([P, H], f32, tag="t3", name="t3")
                    nc.vector.tensor_tensor(
                        t3[:], t2[:], b1_s[:], op=mybir.AluOpType.add)
                    t4 = wp.tile([P, H], bf16, tag="t4", name="t4")
                    nc.scalar.activation(t4[:], t3[:], AF.Relu,
                                         scale=dinv_s[:, wi: wi + 1])
                    nc.sync.dma_start(ht[wi * P: (wi + 1) * P, :], t4[:])

            _agg_core(nc, tc, mybir, plan, table, idx_s, slot_s, iota_s,
                      identb, selfd, (gatp, selp, psump, accp, slabp), drain)
    nc.compile()
    return nc


def _build_k3(plan):
    """out[PADN, C] f32 = log_softmax((dinv*(agg(ht)+self)) @ W2 + b2).

    Ln is batched per group (one [P, n_windows] Ln) to avoid per-window
    Exp<->Ln activation-table reloads.
    """
    bass, bacc, tile, mybir, libcfg, make_identity = _bass_mods()
    ncn, padn, nwin, npair, nchunk, gsz, ngroup, tpad = _derived()
    f32 = mybir.dt.float32
    bf16 = mybir.dt.bfloat16
    AF = mybir.ActivationFunctionType

    nc = bacc.Bacc("TRN2", target_bir_lowering=False, debug=False,
                   num_devices=NCORE)
    w2d = nc.dram_tensor("w2", [H, C], f32, kind="ExternalInput").ap()
    b2d = nc.dram_tensor("b2rep", [P, C], f32, kind="ExternalInput").ap()
    outd = nc.dram_tensor("out", [padn, C], f32, kind="ExternalOutput").ap()

    with tile.TileContext(nc) as tc:
        with (
            tc.tile_pool(name="const", bufs=1) as constp,
            tc.tile_pool(name="gat", bufs=6) as gatp,
            tc.tile_pool(name="sel", bufs=12) as selp,
            tc.tile_pool(name="slab", bufs=8) as slabp,
            tc.tile_pool(name="ps", bufs=1, space="PSUM") as psump,
            tc.tile_pool(name="acc", bufs=2) as accp,
            tc.tile_pool(name="wk", bufs=6) as wp,
            tc.tile_pool(name="grp", bufs=2) as grp,
        ):
            table, idx_s, slot_s, iota_s, dinv_s, selfd, identb = _agg_setup(
                nc, tc, mybir, libcfg, make_identity, constp, plan)
            w2_s = constp.tile([H, C], f32)
            nc.sync.dma_start(w2_s[:], w2d[:, :])
            b2_s = constp.tile([P, C], f32)
            nc.sync.dma_start(b2_s[:], b2d[:, :])
            ident = constp.tile([P, P], f32)
            make_identity(nc, ident[:])

            nwg = 2 * gsz
            gtiles = [None]

            def drain(pin, pr, a0, a1):
                if gtiles[0] is None:
                    gtiles[0] = (
                        grp.tile([P, nwg * C], f32, tag="zbuf", name="zbuf"),
                        grp.tile([P, nwg], f32, tag="negm", name="negm"),
                        grp.tile([P, nwg], f32, tag="sa", name="sa"),
                    )
                zbuf, negm_g, sa_g = gtiles[0]
                for half, a in ((0, a0), (1, a1)):
                    wi = 2 * pr + half
                    wg = 2 * pin + half
                    t1 = wp.tile([P, H], f32, tag="t1", name="t1")
                    nc.scalar.activation(t1[:], a, AF.Copy,
                                         scale=dinv_s[:, wi: wi + 1])
                    t1T_p = psump.tile([H, P], f32, tag="t1T", bufs=1,
                                       name="t1T")
                    nc.tensor.transpose(t1T_p[:], t1[:], ident[:])
                    t1T = wp.tile([H, P], f32, tag="t1Ts", name="t1Ts")
                    nc.scalar.activation(t1T[:], t1T_p[:], AF.Copy)
                    yT_p = psump.tile([C, P], f32, tag="yT", bufs=1,
                                      name="yT")
                    nc.tensor.matmul(yT_p[:], lhsT=w2_s[:], rhs=t1T[:],
                                     start=True, stop=True)
                    yT = wp.tile([C, P], f32, tag="yTs", name="yTs")
                    nc.scalar.activation(yT[:], yT_p[:], AF.Copy)
                    y_p = psump.tile([P, C], f32, tag="y", bufs=1, name="y")
                    nc.tensor.transpose(y_p[:], yT[:], ident[:C, :C])
                    z = zbuf[:, wg * C: (wg + 1) * C]
                    nc.vector.tensor_tensor(z, y_p[:], b2_s[:],
                                            op=mybir.AluOpType.add)
                    nc.vector.tensor_reduce(
                        negm_g[:, wg: wg + 1], z, axis=mybir.AxisListType.X,
                        op=mybir.AluOpType.max, negate=True,
                    )
                    e = wp.tile([P, C], f32, tag="e", name="e")
                    nc.scalar.activation(
                        e[:], z, AF.Exp,
                        bias=negm_g[:, wg: wg + 1],
                        accum_out=sa_g[:, wg: wg + 1],
                    )

            def group_end(g, touched):
                zbuf, negm_g, sa_g = gtiles[0]
                gtiles[0] = None
                lns = wp.tile([P, nwg], f32, tag="lns", name="lns")
                nc.scalar.activation(lns[:], sa_g[:], AF.Ln)
                for (pin, pr) in touched:
                    for half in (0, 1):
                        wi = 2 * pr + half
                        wg = 2 * pin + half
                        o = wp.tile([P, C], f32, tag="o", name="o")
                        nc.vector.tensor_scalar(
                            out=o[:], in0=zbuf[:, wg * C: (wg + 1) * C],
                            scalar1=negm_g[:, wg: wg + 1],
                            scalar2=lns[:, wg: wg + 1],
                            op0=mybir.AluOpType.add,
                            op1=mybir.AluOpType.subtract,
                        )
                        nc.sync.dma_start(outd[wi * P: (wi + 1) * P, :],
                                          o[:])

            _agg_core(nc, tc, mybir, plan, table, idx_s, slot_s, iota_s,
                      identb, selfd, (gatp, selp, psump, accp, slabp), drain,
                      group_end)
    nc.compile()
    return nc


def _run(nc, in_maps):
    if os.environ.get("BASS_GCN_SIM"):
        from concourse.bass_interp import MultiCoreSim

        sim = MultiCoreSim(nc, num_cores=NCORE, trace=False)
        for c in range(NCORE):
            for k, v in in_maps[c].items():
                sim.cores[c].tensor(k)[:] = v
        sim.simulate()
        outs = []
        for c in range(NCORE):
            names = [
                a.memorylocations[0].name
                for a in nc.m.functions[0].allocations
                if getattr(a, "kind", None) == "ExternalOutput"
            ]
            outs.append({n: np.array(sim.cores[c].tensor(n)) for n in names})
        return outs

    from concourse.bass_utils import run_bass_kernel_spmd

    trace = TRACE and _install_ntff_shim()
    res = run_bass_kernel_spmd(nc, in_maps, core_ids=list(range(NCORE)),
                               trace=trace)
    if res.exec_time_ns:
        LAST_EXEC_NS.append(res.exec_time_ns)
    return res.results


# ---------------------------------------------------------------- kernel
def kernel(x, edge_index, W1, b1, W2, b2):
    ncn, padn, nwin, npair, nchunk, gsz, ngroup, tpad = _derived()
    LAST_EXEC_NS.clear()

    x = np.asarray(x, np.float32)
    edge_index = np.asarray(edge_index)
    W1 = np.asarray(W1, np.float32)
    b1 = np.asarray(b1, np.float32)
    W2 = np.asarray(W2, np.float32)
    b2 = np.asarray(b2, np.float32)

    plan = _build_plan(edge_index)

    iota2 = np.tile(np.arange(2 * P, dtype=np.float32)[None, :],
                    (P, 1)).astype(BF16)
    b1rep = np.tile(b1[None, :], (P, 1)).astype(np.float32)
    b2rep = np.tile(b2[None, :], (P, 1)).astype(np.float32)

    # ---- K1
    nw8 = (nwin + 7) // 8
    nwin8 = nw8 * 8
    padn8 = nwin8 * P
    nc1 = _build_k1()
    in1 = []
    for c in range(NCORE):
        xc = np.zeros((padn8, F_IN), np.float32)
        xc[:ncn] = x[c * ncn: (c + 1) * ncn]
        dv8 = np.zeros((P, nwin8), np.float32)
        dv8[:, :nwin] = plan["dinv_w"][c]
        in1.append({
            "xT": np.ascontiguousarray(xc.T).astype(BF16),
            "w1": W1.astype(BF16),
            "dinvw": dv8,
        })
    r1 = _run(nc1, in1)
    ut = np.concatenate([
        np.asarray(r1[c]["ut8"]).reshape(nw8, P, 8, H)
        .transpose(0, 2, 1, 3).reshape(padn8, H)[:ncn]
        for c in range(NCORE)
    ], axis=0)

    def padded_table(t):
        tab = np.zeros((tpad, P), BF16)
        tab[:N, :H] = t
        return tab

    def self_rows(t, c):
        sr = np.zeros((padn, H), BF16)
        lo = c * ncn
        sr[:ncn] = t[lo: lo + ncn]
        return sr

    # ---- K2
    nc2 = _build_k2(plan)
    tab2 = padded_table(ut)
    in2 = [{
        "table": tab2,
        "idx": plan["idxw"][c],
        "slot": plan["slotcols"][c],
        "iota": iota2,
        "dinvw": plan["dinv_w"][c],
        "selfrows": self_rows(ut, c),
        "b1rep": b1rep,
    } for c in range(NCORE)]
    r2 = _run(nc2, in2)
    ht = np.concatenate([r2[c]["ht"][:ncn] for c in range(NCORE)], axis=0)

    # ---- K3
    nc3 = _build_k3(plan)
    tab3 = padded_table(ht)
    in3 = [{
        "table": tab3,
        "idx": plan["idxw"][c],
        "slot": plan["slotcols"][c],
        "iota": iota2,
        "dinvw": plan["dinv_w"][c],
        "selfrows": self_rows(ht, c),
        "w2": W2,
        "b2rep": b2rep,
    } for c in range(NCORE)]
    r3 = _run(nc3, in3)
    out = np.concatenate([r3[c]["out"][:ncn] for c in range(NCORE)], axis=0)
    return np.ascontiguousarray(out.astype(np.float32))



# revision 3
# speedup vs baseline: 1.7273x; 1.0192x over previous
"""GCN (2-layer, symmetric-normalized, self-loops) on 8 TRN2 NeuronCores.

Math (reference):
    A_hat = D^-1/2 (A + I) D^-1/2        (deg over dst incl. self-loops)
    h1    = relu(A_hat @ (x @ W1) + b1)
    out   = log_softmax(A_hat @ h1 @ W2 + b2)

Device decomposition (nodes sharded by range across 8 cores, 3 launches):
    K1: ut   = bf16(dinv * (x @ W1))                     [per-core shard]
    K2: ht   = bf16(dinv*relu(dinv*((A+I) @ ut) + b1))   [gather ut table]
    K3: out  = log_softmax((dinv*((A+I) @ ht)) @ W2 + b2)
Host concatenates shard outputs between launches (index structures are
pure functions of edge_index and are built host-side).

Aggregation: edges (minus self-loops, which are added densely at drain
time) are gathered per (group-of-7-pairs x int16 source chunk) segment
with gpsimd dma_gather from a [N, 128]-padded bf16 table, then
scatter-summed into per-window PSUM accumulators via bf16 one-hot
selection matrices (is_equal against an iota row) on the tensor engine.
Per-(pair,chunk) runs are padded only to the max count over cores
(16-aligned) with valid dummy indices; blocks straddling a pair
boundary issue one matmul pair per touched pair. The gpsimd descriptor
generation (~8ns/idx) is the bottleneck; everything else hides under it.
"""

import math
import os
import sys
import types

import numpy as np
import ml_dtypes

BF16 = ml_dtypes.bfloat16

# ---------------------------------------------------------------- sizes
SMALL = bool(int(os.environ.get("BASS_GCN_SMALL", "0")))
if SMALL:
    N = 4096
    E = 32768
    CHUNK = 1024
else:
    N = 100000
    E = 1600000
    CHUNK = 32768
F_IN = 256
H = 64
C = 16
NCORE = 8
P = 128
TRACE = bool(int(os.environ.get("BASS_GCN_TRACE", "0")))

LAST_EXEC_NS = []        # per-launch exec time (filled when TRACE)


def _derived():
    ncn = N // NCORE
    padn = ((ncn + 255) // 256) * 256
    nwin = padn // P
    npair = nwin // 2
    nchunk = (N + CHUNK - 1) // CHUNK
    # groups of up to 7 pairs
    gsz = 7 if npair >= 7 else npair
    ngroup = (npair + gsz - 1) // gsz
    # padded table rows: cover every core's slab windows
    tpad = ((NCORE - 1) * ncn + padn + P - 1) // P * P
    tpad = max(tpad, N)
    return ncn, padn, nwin, npair, nchunk, gsz, ngroup, tpad


# ------------------------------------------------------- ntff shim (opt)
def _install_ntff_shim():
    try:
        if "antenv.axon_hooks" in sys.modules:
            return True
        sys.path.insert(0, "/root/.axon_site/trn_agent_boot")
        from trn_boot import _ntff_profile_via_ctypes  # type: ignore

        mod = types.ModuleType("antenv.axon_hooks")
        holder = [None]
        mod.set_axon_ntff_profile_hook = lambda h: holder.__setitem__(0, h)
        mod.get_axon_ntff_profile_hook = lambda: holder[0]
        sys.modules["antenv.axon_hooks"] = mod
        import antenv

        antenv.axon_hooks = mod
        mod.set_axon_ntff_profile_hook(
            _ntff_profile_via_ctypes("/opt/axon/libaxon_pjrt.so")
        )
        return True
    except Exception:
        return False


# ------------------------------------------------------------ host plan
def _build_plan(edge_index):
    """Index structures for the per-core edge aggregation (no self-loops).

    Edge order per core: (group, chunk, pair, stable). Per-(pair,chunk)
    runs padded to R[p,c] = 16-align(max over cores of count), with valid
    dummy idx 0 / slot 999. Segment (g,c) = concat of its pairs' runs,
    padded to a 128 multiple.

    Returns dict with:
      nseg_list [ (g,c,S) ... ]          uniform segment sizes
      blocks    [g][c] -> list of (list of (pin, col, start, stop))
      idxw      [NCORE][128, sumS/16] int16   wrapped gather indices
      slotcols  [NCORE][128, ncols] bf16      slot-in-pair per entry col
      dinv_w    [NCORE][128, nwin] f32        dinv per window column
      dinv      [N] f32
    """
    ncn, padn, nwin, npair, nchunk, gsz, ngroup, tpad = _derived()

    src_all = np.asarray(edge_index[0], np.int64)
    dst_all = np.asarray(edge_index[1], np.int64)
    # degree includes self-loop (reference: deg over dst+loop)
    deg = (np.bincount(dst_all, minlength=N) + 1).astype(np.float64)
    dinv = (1.0 / np.sqrt(deg)).astype(np.float32)

    per_core = []
    cnts = np.zeros((NCORE, npair, nchunk), np.int64)
    for c in range(NCORE):
        lo = c * ncn
        m = (dst_all >= lo) & (dst_all < lo + ncn)
        s = src_all[m]
        d = dst_all[m] - lo
        pair = d >> 8
        chunk = s // CHUNK
        grp = pair // gsz
        # sort by (group, chunk, pair), stable
        key = (grp * nchunk + chunk) * npair + pair
        order = np.argsort(key, kind="stable")
        s, d, pair, chunk = s[order], d[order], pair[order], chunk[order]
        np.add.at(cnts[c], (pair, chunk), 1)
        per_core.append((s, d, pair, chunk))

    # segment sizes: pad only the segment total to the max over cores
    # (128-aligned); per-core run boundaries float inside the segment.
    seg_cnt = np.zeros((NCORE, ngroup, nchunk), np.int64)
    for g in range(ngroup):
        p0, p1 = g * gsz, min((g + 1) * gsz, npair)
        seg_cnt[:, g, :] = cnts[:, p0:p1, :].sum(axis=1)
    seg_S = 128 * ((seg_cnt.max(axis=0) + 127) // 128)   # [ngroup, nchunk]

    total = int(seg_S.sum())

    # per-core run offsets within each segment (cumulative, unpadded) for
    # all but the last chunk; the last (tiny) chunk uses uniform padded
    # offsets so same-parity PSUM chain tags can't collide across cores.
    lastc = nchunk - 1
    Rlast = 16 * ((cnts[:, :, lastc].max(axis=0) + 15) // 16)   # [npair]
    run_off = np.zeros((NCORE, npair, nchunk), np.int64)
    for g in range(ngroup):
        p0, p1 = g * gsz, min((g + 1) * gsz, npair)
        for ch in range(nchunk):
            off = np.zeros(NCORE, np.int64)
            for p in range(p0, p1):
                run_off[:, p, ch] = off
                if ch == lastc:
                    off += int(Rlast[p])
                else:
                    off += cnts[:, p, ch]
    seg_cnt[:, :, lastc] = 0
    for g in range(ngroup):
        p0, p1 = g * gsz, min((g + 1) * gsz, npair)
        seg_cnt[:, g, lastc] = int(Rlast[p0:p1].sum())
    seg_S = 128 * ((seg_cnt.max(axis=0) + 127) // 128)
    total = int(seg_S.sum())

    # block descriptors: union over cores of pairs intersecting each block
    blocks = []
    ncols = 0
    for g in range(ngroup):
        p0, p1 = g * gsz, min((g + 1) * gsz, npair)
        gblocks = []
        touches = {p: [] for p in range(p0, p1)}
        per_ch = []
        for ch in range(nchunk):
            S = int(seg_S[g, ch])
            nb = S // 128
            ch_blocks = []
            for b in range(nb):
                lo_e, hi_e = b * 128, (b + 1) * 128
                ents = []
                for p in range(p0, p1):
                    r0 = run_off[:, p, ch]
                    r1 = r0 + cnts[:, p, ch]
                    if ((r0 < hi_e) & (r1 > lo_e)).any():
                        ents.append(p)
                        touches[p].append((ch, b))
                ch_blocks.append(ents)
            per_ch.append(ch_blocks)
        firstch, lastch = {}, {}
        for p in range(p0, p1):
            chs = sorted({ch for (ch, b) in touches[p]})
            if chs:
                firstch[p] = chs[0]
                lastch[p] = chs[-1]
        for ch in range(nchunk):
            out_blocks = []
            pblocks = {}
            for b, ents in enumerate(per_ch[ch]):
                for p in ents:
                    pblocks.setdefault(p, []).append(b)
            for b, ents in enumerate(per_ch[ch]):
                oents = []
                for p in ents:
                    start = pblocks[p][0] == b
                    stop = pblocks[p][-1] == b
                    accfirst = firstch[p] == ch
                    acclast = lastch[p] == ch
                    oents.append((p - p0, ncols, start, stop, p, accfirst,
                                  acclast))
                    ncols += 1
                out_blocks.append(oents)
            gblocks.append(out_blocks)
        blocks.append(gblocks)

    # chain-tag safety: a pair's union interval must not extend past the
    # start block of the next same-parity pair (PSUM tag reuse hazard)
    for g in range(ngroup):
        p0, p1 = g * gsz, min((g + 1) * gsz, npair)
        for ch in range(nchunk):
            lastb = {}
            firstb = {}
            for b, ents in enumerate(blocks[g][ch]):
                for (pin, col, start, stop, p, af, al) in ents:
                    firstb.setdefault(p, b)
                    lastb[p] = b
            for p in range(p0, p1 - 2):
                if p in lastb and (p + 2) in firstb:
                    assert lastb[p] <= firstb[p + 2], (g, ch, p)

    # ---- per-core data arrays
    idxw_l, slot_l, dinvw_l = [], [], []
    seg_base = {}
    off = 0
    for g in range(ngroup):
        for ch in range(nchunk):
            seg_base[(g, ch)] = off
            off += int(seg_S[g, ch])
    for c in range(NCORE):
        s, d, pair, chunk = per_core[c]
        idx16 = np.zeros(total, np.int16)
        slot = np.full(total, 999.0, np.float32)
        grp = pair // gsz
        segid = grp * nchunk + chunk
        key = segid * npair + pair
        uniq, starts = np.unique(key, return_index=True)
        for k, st in zip(uniq, starts):
            p = int(k % npair)
            sg = int(k // npair)
            g, ch = sg // nchunk, sg % nchunk
            cnt = int(cnts[c, p, ch])
            base = seg_base[(g, ch)] + int(run_off[c, p, ch])
            sl = slice(st, st + cnt)
            idx16[base:base + cnt] = (s[sl] % CHUNK).astype(np.int16)
            slot[base:base + cnt] = (d[sl] & 255).astype(np.float32)
        # wrap idx per segment: [S] -> [16, S/16] tiled to 128 rows
        cols16 = []
        for g in range(ngroup):
            for ch in range(nchunk):
                S = int(seg_S[g, ch])
                if S == 0:
                    continue
                a = seg_base[(g, ch)]
                seg = idx16[a:a + S]
                cols16.append(np.tile(seg.reshape(-1, 16).T, (8, 1)))
        idxw_l.append(np.ascontiguousarray(np.concatenate(cols16, axis=1)))

        # slot columns: one column per block entry, masked to the rows of
        # this core's run for that pair
        scols = np.full((P, ncols), 999.0, np.float32)
        for g in range(ngroup):
            p0 = g * gsz
            for ch in range(nchunk):
                a = seg_base[(g, ch)]
                for b, ents in enumerate(blocks[g][ch]):
                    blk_slots = slot[a + b * 128: a + (b + 1) * 128]
                    blk_idx_lo = b * 128
                    for (pin, col, start, stop, p_abs, accfirst,
                         acclast) in ents:
                        r0 = int(run_off[c, p_abs, ch])
                        r1 = r0 + int(cnts[c, p_abs, ch])
                        lo_i = max(r0 - blk_idx_lo, 0)
                        hi_i = min(r1 - blk_idx_lo, P)
                        if lo_i >= hi_i:
                            continue
                        colv = scols[:, col]
                        colv[lo_i:hi_i] = blk_slots[lo_i:hi_i]
        slot_l.append(scols)

        dv = np.zeros((P, nwin), np.float32)
        valid = np.arange(padn) < ncn
        dvfull = np.zeros(padn, np.float32)
        dvfull[:ncn] = dinv[c * ncn: c * ncn + ncn]
        dv[:, :] = dvfull.reshape(nwin, P).T * valid.reshape(nwin, P).T
        dinvw_l.append(dv)

    return {
        "seg_S": seg_S,
        "blocks": blocks,
        "ncols": ncols,
        "total": total,
        "idxw": idxw_l,
        "slotcols": slot_l,
        "dinv_w": dinvw_l,
        "dinv": dinv,
    }


# --------------------------------------------------------- bass builders
def _bass_mods():
    import concourse.bass as bass
    import concourse.bacc as bacc
    import concourse.tile as tile
    import concourse.mybir as mybir
    from concourse import library_config
    from concourse.masks import make_identity

    return bass, bacc, tile, mybir, library_config, make_identity


def _build_k1():
    """ut8[nw8, P, 8*H] bf16 = dinv_col * (x @ W1), 8 windows per DMA.

    Inputs: xT bf16 [F_IN, padn8], w1 bf16 [F_IN, H], dinvw f32 [P, nwin8].
    lhsT = xT tile slice [128f, 128n], rhs = w1 tile [128f, H]. Host
    unscrambles the window-packed output.
    """
    bass, bacc, tile, mybir, libcfg, make_identity = _bass_mods()
    ncn, padn, nwin, npair, nchunk, gsz, ngroup, tpad = _derived()
    nw8 = (nwin + 7) // 8
    nwin8 = nw8 * 8
    padn8 = nwin8 * P
    f32 = mybir.dt.float32
    bf16 = mybir.dt.bfloat16
    AF = mybir.ActivationFunctionType

    nc = bacc.Bacc("TRN2", target_bir_lowering=False, debug=False,
                   num_devices=NCORE)
    xT = nc.dram_tensor("xT", [F_IN, padn8], bf16, kind="ExternalInput").ap()
    w1 = nc.dram_tensor("w1", [F_IN, H], bf16, kind="ExternalInput").ap()
    dinvd = nc.dram_tensor("dinvw", [P, nwin8], f32,
                           kind="ExternalInput").ap()
    ut8 = nc.dram_tensor("ut8", [nw8 * P, 8 * H], bf16,
                         kind="ExternalOutput").ap()

    kf = F_IN // P
    with tile.TileContext(nc) as tc:
        with (
            tc.tile_pool(name="const", bufs=1) as constp,
            tc.tile_pool(name="xin", bufs=3) as xp,
            tc.tile_pool(name="ps", bufs=4, space="PSUM") as psump,
            tc.tile_pool(name="wk", bufs=3) as wp,
        ):
            w1_s = constp.tile([P, kf * H], bf16)
            for k in range(kf):
                nc.sync.dma_start(w1_s[:, k * H: (k + 1) * H],
                                  w1[k * P: (k + 1) * P, :])
            dinv_s = constp.tile([P, nwin8], f32)
            nc.sync.dma_start(dinv_s[:], dinvd[:, :])

            for g8 in range(nw8):
                xg = []
                for k in range(kf):
                    xt = xp.tile([P, 8 * P], bf16, tag=f"xt{k}")
                    nc.sync.dma_start(
                        xt[:],
                        xT[k * P: (k + 1) * P, g8 * 8 * P: (g8 + 1) * 8 * P])
                    xg.append(xt)
                obuf = wp.tile([P, 8 * H], bf16, tag="obuf")
                for j in range(8):
                    t = g8 * 8 + j
                    up = psump.tile([P, H], f32, tag="up", bufs=4)
                    for k in range(kf):
                        nc.tensor.matmul(
                            up[:], lhsT=xg[k][:, j * P: (j + 1) * P],
                            rhs=w1_s[:, k * H: (k + 1) * H],
                            start=(k == 0), stop=(k == kf - 1),
                        )
                    nc.scalar.activation(
                        obuf[:, j * H: (j + 1) * H], up[:], AF.Copy,
                        scale=dinv_s[:, t: t + 1])
                nc.sync.dma_start(ut8[g8 * P: (g8 + 1) * P, :], obuf[:])
    nc.compile()
    return nc


def _agg_core(nc, tc, mybir, plan, table, idx_s, slot_s, iota_s, identb,
              selfd, pools, drain_fn, group_end_fn=None):
    """Shared aggregation: per group, gather 4 chunk segments; per
    (pair, chunk) run a short PSUM accumulation chain (one bank per
    window, <=2 pairs pending at a block boundary); on the pair's last
    chunk append identity-matmuls of the self-loop rows before closing
    the chain; fold finished chains into SBUF accumulators (first: ACT
    copy, later: DVE add); drain per pair, then group_end_fn."""
    f32 = mybir.dt.float32
    bf16 = mybir.dt.bfloat16
    ncn, padn, nwin, npair, nchunk, gsz, ngroup, tpad = _derived()
    gatp, selp, psump, accp, slabp = pools
    seg_S = plan["seg_S"]
    blocks = plan["blocks"]

    off16 = 0
    for g in range(ngroup):
        touched = []
        seen = set()
        for ch in range(nchunk):
            for ents in blocks[g][ch]:
                for e in ents:
                    if e[0] not in seen:
                        seen.add(e[0])
                        touched.append((e[0], e[4]))
        if not touched:
            continue
        acc = {}
        for (pin, p_abs) in touched:
            acc[pin] = accp.tile([P, 2 * H], f32, tag=f"acc{pin}", bufs=2,
                                 name=f"acc{pin}")
        chain = {}
        SUBB = 16            # blocks per sub-gather (2048 idx)
        for ch in range(nchunk):
            S = int(seg_S[g, ch])
            if S == 0:
                continue
            nb = S // P
            lo = ch * CHUNK
            hi = min(tpad, (ch + 1) * CHUNK)
            subs = []
            for o in range(0, nb, SUBB):
                k = min(SUBB, nb - o)
                gsub = gatp.tile([P, k, P], bf16, tag="gat", name="gat")
                nc.gpsimd.dma_gather(
                    gsub[:],
                    table[lo:hi, :],
                    idx_s[:, off16: off16 + k * 8],
                    k * P, k * P, P, elem_step=P, single_packet=False,
                )
                off16 += k * 8
                subs.append(gsub)
            for b in range(nb):
                for (pin, col, start, stop, p_abs, accfirst, acclast) in \
                        blocks[g][ch][b]:
                    if start:
                        chain[pin] = (
                            psump.tile([P, H], f32, tag=f"cw{pin % 2}0",
                                       bufs=1, name=f"cw{pin % 2}0"),
                            psump.tile([P, H], f32, tag=f"cw{pin % 2}1",
                                       bufs=1, name=f"cw{pin % 2}1"),
                        )
                    sel2 = selp.tile([P, 2 * P], bf16, tag="sel2",
                                     name="sel2")
                    nc.vector.tensor_tensor(
                        out=sel2[:],
                        in0=slot_s[:, col: col + 1].to_broadcast([P, 2 * P]),
                        in1=iota_s[:],
                        op=mybir.AluOpType.is_equal,
                    )
                    c0, c1 = chain[pin]
                    rhs = subs[b // SUBB][:, b % SUBB, :H]
                    last_mm = stop and not acclast
                    nc.tensor.matmul(
                        c0[:], lhsT=sel2[:, :P], rhs=rhs,
                        start=start, stop=last_mm,
                    )
                    nc.tensor.matmul(
                        c1[:], lhsT=sel2[:, P:], rhs=rhs,
                        start=start, stop=last_mm,
                    )
                    if stop:
                        if acclast:
                            # append self-loop rows, closing the chains
                            for half, cx in ((0, c0), (1, c1)):
                                wi = 2 * p_abs + half
                                sl = slabp.tile([P, H], bf16, tag="sl",
                                                name="sl")
                                nc.sync.dma_start(
                                    sl[:], selfd[wi * P:(wi + 1) * P, :])
                                nc.tensor.matmul(
                                    cx[:], lhsT=identb[:], rhs=sl[:],
                                    start=False, stop=True,
                                )
                        a = acc[pin]
                        if accfirst:
                            nc.scalar.activation(
                                a[:, :H], c0[:],
                                mybir.ActivationFunctionType.Copy)
                            nc.scalar.activation(
                                a[:, H:], c1[:],
                                mybir.ActivationFunctionType.Copy)
                        else:
                            nc.vector.tensor_tensor(
                                a[:, :H], a[:, :H], c0[:],
                                op=mybir.AluOpType.add)
                            nc.vector.tensor_tensor(
                                a[:, H:], a[:, H:], c1[:],
                                op=mybir.AluOpType.add)
        for (pin, p_abs) in touched:
            a = acc[pin]
            drain_fn(pin, p_abs, a[:, :H], a[:, H:])
        if group_end_fn is not None:
            group_end_fn(g, touched)


def _agg_setup(nc, tc, mybir, libcfg, make_identity, constp, plan):
    """Load shared aggregation constants."""
    f32 = mybir.dt.float32
    bf16 = mybir.dt.bfloat16
    ncn, padn, nwin, npair, nchunk, gsz, ngroup, tpad = _derived()
    idx_cols = plan["idxw"][0].shape[1]
    ncols = plan["ncols"]

    table = nc.dram_tensor("table", [tpad, P], bf16, kind="ExternalInput").ap()
    idxd = nc.dram_tensor("idx", [P, idx_cols], mybir.dt.int16,
                          kind="ExternalInput").ap()
    slotd = nc.dram_tensor("slot", [P, ncols], f32,
                           kind="ExternalInput").ap()
    iotad = nc.dram_tensor("iota", [P, 2 * P], bf16, kind="ExternalInput").ap()
    dinvd = nc.dram_tensor("dinvw", [P, nwin], f32, kind="ExternalInput").ap()
    selfd = nc.dram_tensor("selfrows", [padn, H], bf16,
                           kind="ExternalInput").ap()

    with tc.tile_critical():
        nc.gpsimd.load_library(libcfg.mlp)
    idx_s = constp.tile([P, idx_cols], mybir.dt.int16)
    nc.sync.dma_start(idx_s[:], idxd[:, :])
    slot_s = constp.tile([P, ncols], f32)
    nc.sync.dma_start(slot_s[:], slotd[:, :])
    iota_s = constp.tile([P, 2 * P], bf16)
    nc.sync.dma_start(iota_s[:], iotad[:, :])
    dinv_s = constp.tile([P, nwin], f32)
    nc.sync.dma_start(dinv_s[:], dinvd[:, :])
    identb = constp.tile([P, P], bf16)
    make_identity(nc, identb[:])
    return table, idx_s, slot_s, iota_s, dinv_s, selfd, identb


def _build_k2(plan):
    """ht[PADN, H] bf16 = dinv*relu(dinv*(agg(ut)+self) + b1) per core."""
    bass, bacc, tile, mybir, libcfg, make_identity = _bass_mods()
    ncn, padn, nwin, npair, nchunk, gsz, ngroup, tpad = _derived()
    f32 = mybir.dt.float32
    bf16 = mybir.dt.bfloat16
    AF = mybir.ActivationFunctionType

    nc = bacc.Bacc("TRN2", target_bir_lowering=False, debug=False,
                   num_devices=NCORE)
    b1d = nc.dram_tensor("b1rep", [P, H], f32, kind="ExternalInput").ap()
    ht = nc.dram_tensor("ht", [padn, H], bf16, kind="ExternalOutput").ap()

    with tile.TileContext(nc) as tc:
        with (
            tc.tile_pool(name="const", bufs=1) as constp,
            tc.tile_pool(name="gat", bufs=10) as gatp,
            tc.tile_pool(name="sel", bufs=12) as selp,
            tc.tile_pool(name="slab", bufs=8) as slabp,
            tc.tile_pool(name="ps", bufs=1, space="PSUM") as psump,
            tc.tile_pool(name="acc", bufs=2) as accp,
            tc.tile_pool(name="wk", bufs=6) as wp,
        ):
            table, idx_s, slot_s, iota_s, dinv_s, selfd, identb = _agg_setup(
                nc, tc, mybir, libcfg, make_identity, constp, plan)
            b1_s = constp.tile([P, H], f32)
            nc.sync.dma_start(b1_s[:], b1d[:, :])

            def drain(pin, pr, a0, a1):
                for wi, a in ((2 * pr, a0), (2 * pr + 1, a1)):
                    t2 = wp.tile([P, H], f32, tag="t2", name="t2")
                    nc.scalar.activation(t2[:], a, AF.Copy,
                                         scale=dinv_s[:, wi: wi + 1])
                    t3 = wp.tile([P, H], f32, tag="t3", name="t3")
                    nc.vector.tensor_tensor(
                        t3[:], t2[:], b1_s[:], op=mybir.AluOpType.add)
                    t4 = wp.tile([P, H], bf16, tag="t4", name="t4")
                    nc.scalar.activation(t4[:], t3[:], AF.Relu,
                                         scale=dinv_s[:, wi: wi + 1])
                    nc.sync.dma_start(ht[wi * P: (wi + 1) * P, :], t4[:])

            _agg_core(nc, tc, mybir, plan, table, idx_s, slot_s, iota_s,
                      identb, selfd, (gatp, selp, psump, accp, slabp), drain)
    nc.compile()
    return nc


def _build_k3(plan):
    """out[PADN, C] f32 = log_softmax((dinv*(agg(ht)+self)) @ W2 + b2).

    Ln is batched per group (one [P, n_windows] Ln) to avoid per-window
    Exp<->Ln activation-table reloads.
    """
    bass, bacc, tile, mybir, libcfg, make_identity = _bass_mods()
    ncn, padn, nwin, npair, nchunk, gsz, ngroup, tpad = _derived()
    f32 = mybir.dt.float32
    bf16 = mybir.dt.bfloat16
    AF = mybir.ActivationFunctionType

    nc = bacc.Bacc("TRN2", target_bir_lowering=False, debug=False,
                   num_devices=NCORE)
    w2d = nc.dram_tensor("w2", [H, C], f32, kind="ExternalInput").ap()
    b2d = nc.dram_tensor("b2rep", [P, C], f32, kind="ExternalInput").ap()
    outd = nc.dram_tensor("out", [padn, C], f32, kind="ExternalOutput").ap()

    with tile.TileContext(nc) as tc:
        with (
            tc.tile_pool(name="const", bufs=1) as constp,
            tc.tile_pool(name="gat", bufs=10) as gatp,
            tc.tile_pool(name="sel", bufs=12) as selp,
            tc.tile_pool(name="slab", bufs=8) as slabp,
            tc.tile_pool(name="ps", bufs=1, space="PSUM") as psump,
            tc.tile_pool(name="acc", bufs=2) as accp,
            tc.tile_pool(name="wk", bufs=6) as wp,
            tc.tile_pool(name="grp", bufs=2) as grp,
        ):
            table, idx_s, slot_s, iota_s, dinv_s, selfd, identb = _agg_setup(
                nc, tc, mybir, libcfg, make_identity, constp, plan)
            w2_s = constp.tile([H, C], f32)
            nc.sync.dma_start(w2_s[:], w2d[:, :])
            b2_s = constp.tile([P, C], f32)
            nc.sync.dma_start(b2_s[:], b2d[:, :])
            ident = constp.tile([P, P], f32)
            make_identity(nc, ident[:])

            nwg = 2 * gsz
            gtiles = [None]

            def drain(pin, pr, a0, a1):
                if gtiles[0] is None:
                    gtiles[0] = (
                        grp.tile([P, nwg * C], f32, tag="zbuf", name="zbuf"),
                        grp.tile([P, nwg], f32, tag="negm", name="negm"),
                        grp.tile([P, nwg], f32, tag="sa", name="sa"),
                    )
                zbuf, negm_g, sa_g = gtiles[0]
                for half, a in ((0, a0), (1, a1)):
                    wi = 2 * pr + half
                    wg = 2 * pin + half
                    t1 = wp.tile([P, H], f32, tag="t1", name="t1")
                    nc.scalar.activation(t1[:], a, AF.Copy,
                                         scale=dinv_s[:, wi: wi + 1])
                    t1T_p = psump.tile([H, P], f32, tag="t1T", bufs=1,
                                       name="t1T")
                    nc.tensor.transpose(t1T_p[:], t1[:], ident[:])
                    t1T = wp.tile([H, P], f32, tag="t1Ts", name="t1Ts")
                    nc.scalar.activation(t1T[:], t1T_p[:], AF.Copy)
                    yT_p = psump.tile([C, P], f32, tag="yT", bufs=1,
                                      name="yT")
                    nc.tensor.matmul(yT_p[:], lhsT=w2_s[:], rhs=t1T[:],
                                     start=True, stop=True)
                    yT = wp.tile([C, P], f32, tag="yTs", name="yTs")
                    nc.scalar.activation(yT[:], yT_p[:], AF.Copy)
                    y_p = psump.tile([P, C], f32, tag="y", bufs=1, name="y")
                    nc.tensor.transpose(y_p[:], yT[:], ident[:C, :C])
                    z = zbuf[:, wg * C: (wg + 1) * C]
                    nc.vector.tensor_tensor(z, y_p[:], b2_s[:],
                                            op=mybir.AluOpType.add)
                    nc.vector.tensor_reduce(
                        negm_g[:, wg: wg + 1], z, axis=mybir.AxisListType.X,
                        op=mybir.AluOpType.max, negate=True,
                    )
                    e = wp.tile([P, C], f32, tag="e", name="e")
                    nc.scalar.activation(
                        e[:], z, AF.Exp,
                        bias=negm_g[:, wg: wg + 1],
                        accum_out=sa_g[:, wg: wg + 1],
                    )

            def group_end(g, touched):
                zbuf, negm_g, sa_g = gtiles[0]
                gtiles[0] = None
                lns = wp.tile([P, nwg], f32, tag="lns", name="lns")
                nc.scalar.activation(lns[:], sa_g[:], AF.Ln)
                for (pin, pr) in touched:
                    for half in (0, 1):
                        wi = 2 * pr + half
                        wg = 2 * pin + half
                        o = wp.tile([P, C], f32, tag="o", name="o")
                        nc.vector.tensor_scalar(
                            out=o[:], in0=zbuf[:, wg * C: (wg + 1) * C],
                            scalar1=negm_g[:, wg: wg + 1],
                            scalar2=lns[:, wg: wg + 1],
                            op0=mybir.AluOpType.add,
                            op1=mybir.AluOpType.subtract,
                        )
                        nc.sync.dma_start(outd[wi * P: (wi + 1) * P, :],
                                          o[:])

            _agg_core(nc, tc, mybir, plan, table, idx_s, slot_s, iota_s,
                      identb, selfd, (gatp, selp, psump, accp, slabp), drain,
                      group_end)
    nc.compile()
    return nc


def _run(nc, in_maps):
    if os.environ.get("BASS_GCN_SIM"):
        from concourse.bass_interp import MultiCoreSim

        sim = MultiCoreSim(nc, num_cores=NCORE, trace=False)
        for c in range(NCORE):
            for k, v in in_maps[c].items():
                sim.cores[c].tensor(k)[:] = v
        sim.simulate()
        outs = []
        for c in range(NCORE):
            names = [
                a.memorylocations[0].name
                for a in nc.m.functions[0].allocations
                if getattr(a, "kind", None) == "ExternalOutput"
            ]
            outs.append({n: np.array(sim.cores[c].tensor(n)) for n in names})
        return outs

    from concourse.bass_utils import run_bass_kernel_spmd

    trace = TRACE and _install_ntff_shim()
    res = run_bass_kernel_spmd(nc, in_maps, core_ids=list(range(NCORE)),
                               trace=trace)
    if res.exec_time_ns:
        LAST_EXEC_NS.append(res.exec_time_ns)
    return res.results


# ---------------------------------------------------------------- kernel
def kernel(x, edge_index, W1, b1, W2, b2):
    ncn, padn, nwin, npair, nchunk, gsz, ngroup, tpad = _derived()
    LAST_EXEC_NS.clear()

    x = np.asarray(x, np.float32)
    edge_index = np.asarray(edge_index)
    W1 = np.asarray(W1, np.float32)
    b1 = np.asarray(b1, np.float32)
    W2 = np.asarray(W2, np.float32)
    b2 = np.asarray(b2, np.float32)

    plan = _build_plan(edge_index)

    iota2 = np.tile(np.arange(2 * P, dtype=np.float32)[None, :],
                    (P, 1)).astype(BF16)
    b1rep = np.tile(b1[None, :], (P, 1)).astype(np.float32)
    b2rep = np.tile(b2[None, :], (P, 1)).astype(np.float32)

    # ---- K1
    nw8 = (nwin + 7) // 8
    nwin8 = nw8 * 8
    padn8 = nwin8 * P
    nc1 = _build_k1()
    in1 = []
    for c in range(NCORE):
        xc = np.zeros((padn8, F_IN), np.float32)
        xc[:ncn] = x[c * ncn: (c + 1) * ncn]
        dv8 = np.zeros((P, nwin8), np.float32)
        dv8[:, :nwin] = plan["dinv_w"][c]
        in1.append({
            "xT": np.ascontiguousarray(xc.T).astype(BF16),
            "w1": W1.astype(BF16),
            "dinvw": dv8,
        })
    r1 = _run(nc1, in1)
    ut = np.concatenate([
        np.asarray(r1[c]["ut8"]).reshape(nw8, P, 8, H)
        .transpose(0, 2, 1, 3).reshape(padn8, H)[:ncn]
        for c in range(NCORE)
    ], axis=0)

    def padded_table(t):
        tab = np.zeros((tpad, P), BF16)
        tab[:N, :H] = t
        return tab

    def self_rows(t, c):
        sr = np.zeros((padn, H), BF16)
        lo = c * ncn
        sr[:ncn] = t[lo: lo + ncn]
        return sr

    # ---- K2
    nc2 = _build_k2(plan)
    tab2 = padded_table(ut)
    in2 = [{
        "table": tab2,
        "idx": plan["idxw"][c],
        "slot": plan["slotcols"][c],
        "iota": iota2,
        "dinvw": plan["dinv_w"][c],
        "selfrows": self_rows(ut, c),
        "b1rep": b1rep,
    } for c in range(NCORE)]
    r2 = _run(nc2, in2)
    ht = np.concatenate([r2[c]["ht"][:ncn] for c in range(NCORE)], axis=0)

    # ---- K3
    nc3 = _build_k3(plan)
    tab3 = padded_table(ht)
    in3 = [{
        "table": tab3,
        "idx": plan["idxw"][c],
        "slot": plan["slotcols"][c],
        "iota": iota2,
        "dinvw": plan["dinv_w"][c],
        "selfrows": self_rows(ht, c),
        "w2": W2,
        "b2rep": b2rep,
    } for c in range(NCORE)]
    r3 = _run(nc3, in3)
    out = np.concatenate([r3[c]["out"][:ncn] for c in range(NCORE)], axis=0)
    return np.ascontiguousarray(out.astype(np.float32))

